# revision 1
# baseline (speedup 1.0000x reference)
"""Trainium2 Bass kernel for nn_ConvAlignLoss (8-core data parallel).

Self-contained: hardcodes shapes; imports concourse from /opt/trn_rl_repo.

Per core (R=64 rows):
  loss_astf partial: sum((pred-true)^2)
  conv = irfft16384(fft(pred) * conj(fft(egf_pad)))[:14337]  (2-stage matmul FFT)
  cc   = irfft32768(fft(conv_pad) * conj(fft(target_pad)))
  shift = mapped masked argmax of cc (== reference argmax over n=28673)
  loss_conv partial: sum((conv[(7040+i+shift) % 14337] - target[7040+i])^2)
Host combines the 8 cores' (sum_astf, sum_conv) into the scalar losses.

FFT structure (N = 128*N2):
  FWD:  D[t1,t2]=x[N2*t1+t2]; A[t2,f1]=sum_t1 D*W1 (data-stationary matmul);
        B=A*tw; Z[f2,f1]=sum_t2 W2[t2,f2]*B.   Z2d[f2,f1] == X[f1+128*f2]
  INV:  G[f1,t2]=sum_f2 S2d[f2,f1]*V2[f2,t2] (S stationary); H=G*itw;
        x2d[t1,t2]=(1/N) Re(sum_f1 V1[f1,t1]*H[f1,t2])
"""
import sys

sys.path.insert(0, "/opt/trn_rl_repo")

import numpy as np
import concourse.bass as bass
import concourse.bacc as bacc
import concourse.mybir as mybir
from concourse import tile

F32 = mybir.dt.float32
BF16 = mybir.dt.bfloat16
I32 = mybir.dt.int32
AT = mybir.AluOpType
AX = mybir.AxisListType

R = 64
NCORES = 8
L1, L2 = 16384, 2048
CONV_LEN = L1 - L2 + 1      # 14337
N_A, N_B = 16384, 32768
GAP_LO, GAP_HI = CONV_LEN, N_B - CONV_LEN + 1   # gap [14337, 18432)
CROP = 256
START0 = (CONV_LEN - CROP) // 2                 # 7040
PITCH = 14720
BIGL = float(2 ** 23)


def _dft(n, sign):
    k = np.arange(n)
    return np.exp(sign * 2j * np.pi * np.outer(k, k) / n)


def make_consts():
    c = {}

    def put(name, arr, dt=np.float32):
        c[name] = np.ascontiguousarray(np.asarray(arr, np.float64)).astype(dt)

    W1 = _dft(128, -1)
    put("W1r", W1.real); put("W1i", W1.imag); put("nW1i", -W1.imag)
    twA = np.exp(-2j * np.pi * np.outer(np.arange(128), np.arange(128)) / N_A)
    put("twAr", twA.real); put("twAi", twA.imag)
    V2A = _dft(128, +1)
    put("V2Ar", V2A.real); put("V2Ai", V2A.imag); put("nV2Ai", -V2A.imag)
    itwA = np.exp(2j * np.pi * np.outer(np.arange(128), np.arange(128)) / N_A)
    put("itwAr", itwA.real); put("itwAi", itwA.imag)
    V1A = _dft(128, +1) / N_A
    put("V1Ar", V1A.real); put("nV1Ai", -V1A.imag)

    W2B = _dft(256, -1)          # [t2, f2]
    for a in range(2):
        for b in range(2):
            blk = W2B[a * 128:(a + 1) * 128, b * 128:(b + 1) * 128]
            put(f"W2Br{a}{b}", blk.real)
            put(f"W2Bi{a}{b}", blk.imag)
            put(f"nW2Bi{a}{b}", -blk.imag)
    twB = np.exp(-2j * np.pi * np.outer(np.arange(256), np.arange(128)) / N_B)
    for a in range(2):
        put(f"twBr{a}", twB.real[a * 128:(a + 1) * 128])
        put(f"twBi{a}", twB.imag[a * 128:(a + 1) * 128])
    V2B = _dft(256, +1)          # [f2, t2]
    for a in range(2):
        blk = V2B[a * 128:(a + 1) * 128, :]
        put(f"V2Br{a}", blk.real)
        put(f"V2Bi{a}", blk.imag)
        put(f"nV2Bi{a}", -blk.imag)
    itwB = np.exp(2j * np.pi * np.outer(np.arange(128), np.arange(256)) / N_B)
    put("itwBr", itwB.real); put("itwBi", itwB.imag)
    V1B = _dft(128, +1) / N_B
    put("V1Br", V1B.real); put("nV1Bi", -V1B.imag)

    put("ident", np.eye(128))
    put("ones1x128", np.ones((1, 128)))
    put("ones128", np.ones((128, 1)))
    put("ones64", np.ones((64, 1)))

    j = np.arange(128)[:, None] * 256 + np.arange(256)[None, :]   # [t1, t2]
    gap = (j >= GAP_LO) & (j < GAP_HI)
    put("maskB", np.where(gap, -1e30, 0.0))
    shiftval = np.where(j <= CONV_LEN - 1, j - (CONV_LEN - 1), j - GAP_HI + 1)
    put("shvB", np.where(gap, 0.0, shiftval - BIGL))
    put("winidx", np.arange(R)[:, None] * PITCH
        + np.arange(CROP)[None, :])                               # [64, 256]
    return c


def _b3(ap, n, inner):
    """[128, inner] const AP -> [128, n, inner] broadcast over middle dim."""
    return ap.rearrange("p (a b) -> p a b", a=1).to_broadcast([128, n, inner])


def _cmul_psum(nc, pool, tag, outr, outi, pr, pi, twr, twi, inner, n):
    """(outr + i outi) = (pr + i pi) * (twr + i twi); p* in PSUM, tw const APs
    broadcast over n blocks of `inner`. outr/outi are SBUF APs [128, n*inner]."""
    tmp = pool.tile([128, n * inner], F32, tag="twtmp", name="twtmp")
    orv = outr.rearrange("p (a b) -> p a b", b=inner)
    oiv = outi.rearrange("p (a b) -> p a b", b=inner)
    prv = pr.rearrange("p (a b) -> p a b", b=inner)
    piv = pi.rearrange("p (a b) -> p a b", b=inner)
    tv = tmp[:].rearrange("p (a b) -> p a b", b=inner)
    nc.vector.tensor_tensor(orv, prv, twr, op=AT.mult)
    nc.vector.tensor_tensor(tv, piv, twi, op=AT.mult)
    nc.vector.tensor_tensor(orv, orv, tv, op=AT.subtract)
    nc.vector.tensor_tensor(oiv, prv, twi, op=AT.mult)
    nc.vector.tensor_tensor(tv, piv, twr, op=AT.mult)
    nc.vector.tensor_tensor(oiv, oiv, tv, op=AT.add)


def _mm_const_names():
    s = {"W1r", "W1i", "nW1i", "V2Ar", "V2Ai", "nV2Ai", "V1Ar", "nV1Ai",
         "itwAr", "itwAi", "twAr", "twAi", "itwBr", "itwBi", "V1Br",
         "nV1Bi", "ident"}
    s |= {f"W2Br{a}{b}" for a in range(2) for b in range(2)}
    s |= {f"W2Bi{a}{b}" for a in range(2) for b in range(2)}
    s |= {f"nW2Bi{a}{b}" for a in range(2) for b in range(2)}
    s |= {f"twBr{a}" for a in range(2)} | {f"twBi{a}" for a in range(2)}
    s |= {f"V2Br{a}" for a in range(2)} | {f"V2Bi{a}" for a in range(2)}
    s |= {f"nV2Bi{a}" for a in range(2)}
    return s


def build_nc(cdt=F32, rows=R, rbb=8, rb2=4):
    nc = bacc.Bacc("TRN2", target_bir_lowering=False, debug=False,
                   num_devices=NCORES)
    consts = make_consts()

    pred = nc.dram_tensor("pred", [rows, L1], F32, kind="ExternalInput")
    true_ = nc.dram_tensor("true", [rows, L1], F32, kind="ExternalInput")
    egf = nc.dram_tensor("egf", [rows, L2], F32, kind="ExternalInput")
    target = nc.dram_tensor("target", [rows, CONV_LEN], F32,
                            kind="ExternalInput")
    out = nc.dram_tensor("out", [1, 2], F32, kind="ExternalOutput")
    scratch = nc.dram_tensor("scratch", [rows, PITCH], F32)

    MM_CONST = _mm_const_names()

    cdram = {}
    for name, arr in consts.items():
        cdt_n = cdt if name in MM_CONST else F32
        cdram[name] = nc.dram_tensor(name, list(arr.shape), cdt_n,
                                     kind="ExternalInput")

    nb1, nb2 = rows // rbb, rows // rb2

    with tile.TileContext(nc) as tc:
        with (
            tc.tile_pool(name="consts", bufs=1) as cpool,
            tc.tile_pool(name="keep", bufs=1) as kpool,
            tc.tile_pool(name="ps", bufs=2, space="PSUM") as pp,
        ):
            cs = {}
            for name, arr in consts.items():
                dt = cdt if name in MM_CONST else F32
                t = cpool.tile(list(arr.shape), dt, tag=f"c_{name}", name=f"c_{name}")
                nc.sync.dma_start(t[:], cdram[name][:])
                cs[name] = t

            allmax = kpool.tile([128, rows], F32, tag="allmax", name="allmax")
            allmin = kpool.tile([128, rows], F32, tag="allmin", name="allmin")
            ccm_all = kpool.tile([128, rows * 256], BF16, tag="ccm", name="ccm")
            astf_acc = kpool.tile([128, 8], F32, tag="astfacc", name="astfacc")
            shifts = kpool.tile([rows, 1], F32, tag="shifts", name="shifts")
            outt = kpool.tile([1, 2], F32, tag="outt", name="outt")

            # ---------------- A) astf ----------------
            predf = pred.ap().rearrange("r l -> (r l)").rearrange(
                "(p f) -> p f", p=128)
            truef = true_.ap().rearrange("r l -> (r l)").rearrange(
                "(p f) -> p f", p=128)
            fch = rows * L1 // 128 // 8
            with tc.tile_pool(name="astf", bufs=2) as apool:
                for i in range(8):
                    tp = apool.tile([128, fch], F32, tag="ap", name="ap")
                    tt = apool.tile([128, fch], F32, tag="at", name="at")
                    sl = bass.ts(i, fch)
                    nc.sync.dma_start(tp[:], predf[:, sl])
                    nc.sync.dma_start(tt[:], truef[:, sl])
                    nc.vector.tensor_tensor(tt[:], tp[:], tt[:], op=AT.subtract)
                    nc.vector.scalar_tensor_tensor(
                        tp[:], tt[:], 1.0, tt[:], op0=AT.bypass, op1=AT.mult,
                        accum_out=astf_acc[:, i:i + 1])

            # ---------------- B) 16K level ----------------
            with tc.tile_pool(name="p16", bufs=1) as dp:
                for b in range(nb1):
                    r0 = b * rbb
                    Dp = dp.tile([128, rbb * 128], cdt, tag="Dp", name="Dp")
                    De = dp.tile([128, rbb * 128], cdt, tag="De", name="De")
                    nc.scalar.memzero(De[:])
                    if cdt == F32:
                        for q in range(rbb):
                            r = r0 + q
                            nc.sync.dma_start(
                                Dp[:, bass.ts(q, 128)],
                                pred[r, :].rearrange("(a b) -> a b", a=128))
                            nc.sync.dma_start(
                                De[:16, bass.ts(q, 128)],
                                egf[r, :].rearrange("(a b) -> a b", a=16))
                    else:
                        Dst = dp.tile([128, rbb * 128], F32, tag="Dst", name="Dst")
                        Est = dp.tile([16, rbb * 128], F32, tag="Est", name="Est")
                        for q in range(rbb):
                            r = r0 + q
                            nc.sync.dma_start(
                                Dst[:, bass.ts(q, 128)],
                                pred[r, :].rearrange("(a b) -> a b", a=128))
                            nc.sync.dma_start(
                                Est[:16, bass.ts(q, 128)],
                                egf[r, :].rearrange("(a b) -> a b", a=16))
                        nc.scalar.copy(Dp[:], Dst[:])
                        nc.scalar.copy(De[:16, :], Est[:16, :])

                    Bs = {k: dp.tile([128, rbb * 128], cdt, tag=f"B{k}", name=f"B{k}")
                          for k in ("pr", "pi", "er", "ei")}
                    for g in range(rbb // 4):
                        gsl = bass.ts(g, 512)
                        for inp, D in (("p", Dp), ("e", De)):
                            pa = pp.tile([128, 512], F32, tag="st1", name="st1")
                            pai = pp.tile([128, 512], F32, tag="st1", name="st1")
                            for q in range(4):
                                qq = g * 4 + q
                                sl, osl = bass.ts(qq, 128), bass.ts(q, 128)
                                nc.tensor.matmul(pa[:, osl], lhsT=D[:, sl],
                                                 rhs=cs["W1r"][:],
                                                 start=True, stop=True)
                                nc.tensor.matmul(pai[:, osl], lhsT=D[:, sl],
                                                 rhs=cs["W1i"][:],
                                                 start=True, stop=True)
                            _cmul_psum(nc, dp, "tw",
                                       Bs[inp + "r"][:, gsl], Bs[inp + "i"][:, gsl],
                                       pa[:], pai[:],
                                       _b3(cs["twAr"][:], 4, 128),
                                       _b3(cs["twAi"][:], 4, 128), 128, 4)

                    Zs = {k: dp.tile([128, rbb * 128], cdt, tag=f"Z{k}", name=f"Z{k}")
                          for k in ("pr", "pi", "er", "ei")}
                    for g in range(rbb // 4):
                        gsl = bass.ts(g, 512)
                        for inp in ("p", "e"):
                            br, bi = Bs[inp + "r"], Bs[inp + "i"]
                            pzr = pp.tile([128, 512], F32, tag="st2", name="st2")
                            pzi = pp.tile([128, 512], F32, tag="st2", name="st2")
                            nc.tensor.matmul(pzr[:], lhsT=cs["W1r"][:],
                                             rhs=br[:, gsl], start=True, stop=False)
                            nc.tensor.matmul(pzr[:], lhsT=cs["nW1i"][:],
                                             rhs=bi[:, gsl], start=False, stop=True)
                            nc.tensor.matmul(pzi[:], lhsT=cs["W1i"][:],
                                             rhs=br[:, gsl], start=True, stop=False)
                            nc.tensor.matmul(pzi[:], lhsT=cs["W1r"][:],
                                             rhs=bi[:, gsl], start=False, stop=True)
                            nc.scalar.copy(Zs[inp + "r"][:, gsl], pzr[:])
                            nc.scalar.copy(Zs[inp + "i"][:, gsl], pzi[:])

                    Sr = dp.tile([128, rbb * 128], cdt, tag="Sr", name="Sr")
                    Si = dp.tile([128, rbb * 128], cdt, tag="Si", name="Si")
                    tmpb = dp.tile([128, rbb * 128], F32, tag="tmpbig", name="tmpbig")
                    nc.vector.tensor_tensor(Sr[:], Zs["pr"][:], Zs["er"][:], op=AT.mult)
                    nc.vector.tensor_tensor(tmpb[:], Zs["pi"][:], Zs["ei"][:], op=AT.mult)
                    nc.vector.tensor_tensor(Sr[:], Sr[:], tmpb[:], op=AT.add)
                    nc.vector.tensor_tensor(Si[:], Zs["pi"][:], Zs["er"][:], op=AT.mult)
                    nc.vector.tensor_tensor(tmpb[:], Zs["pr"][:], Zs["ei"][:], op=AT.mult)
                    nc.vector.tensor_tensor(Si[:], Si[:], tmpb[:], op=AT.subtract)

                    Hr = dp.tile([128, rbb * 128], cdt, tag="Hr", name="Hr")
                    Hi = dp.tile([128, rbb * 128], cdt, tag="Hi", name="Hi")
                    for g in range(rbb // 4):
                        gsl = bass.ts(g, 512)
                        pgr = pp.tile([128, 512], F32, tag="inv", name="inv")
                        pgi = pp.tile([128, 512], F32, tag="inv", name="inv")
                        for q in range(4):
                            qq = g * 4 + q
                            sl, osl = bass.ts(qq, 128), bass.ts(q, 128)
                            nc.tensor.matmul(pgr[:, osl], lhsT=Sr[:, sl],
                                             rhs=cs["V2Ar"][:], start=True, stop=False)
                            nc.tensor.matmul(pgr[:, osl], lhsT=Si[:, sl],
                                             rhs=cs["nV2Ai"][:], start=False, stop=True)
                            nc.tensor.matmul(pgi[:, osl], lhsT=Sr[:, sl],
                                             rhs=cs["V2Ai"][:], start=True, stop=False)
                            nc.tensor.matmul(pgi[:, osl], lhsT=Si[:, sl],
                                             rhs=cs["V2Ar"][:], start=False, stop=True)
                        _cmul_psum(nc, dp, "tw", Hr[:, gsl], Hi[:, gsl],
                                   pgr[:], pgi[:],
                                   _b3(cs["itwAr"][:], 4, 128),
                                   _b3(cs["itwAi"][:], 4, 128), 128, 4)

                    convSB = dp.tile([128, rbb * 128], F32, tag="convSB", name="convSB")
                    for g in range(rbb // 4):
                        gsl = bass.ts(g, 512)
                        pc = pp.tile([128, 512], F32, tag="cc", name="cc")
                        nc.tensor.matmul(pc[:], lhsT=cs["V1Ar"][:],
                                         rhs=Hr[:, gsl], start=True, stop=False)
                        nc.tensor.matmul(pc[:], lhsT=cs["nV1Ai"][:],
                                         rhs=Hi[:, gsl], start=False, stop=True)
                        nc.scalar.copy(convSB[:, gsl], pc[:])

                    for q in range(rbb):
                        r = r0 + q
                        csl = bass.ts(q, 128)
                        nc.sync.dma_start(
                            scratch[r, 0:14336].rearrange("(a b) -> a b", a=112),
                            convSB[0:112, csl])
                        nc.sync.dma_start(
                            scratch[r, 14336:14337].rearrange("(a b) -> a b", a=1),
                            convSB[112:113, q * 128:q * 128 + 1])
                        nc.sync.dma_start(
                            scratch[r, 14337:14593].rearrange("(a b) -> a b", a=2),
                            convSB[0:2, csl])

            # ---------------- C) 32K level ----------------
            with tc.tile_pool(name="p32", bufs=1) as dp:
                for b in range(nb2):
                    r0 = b * rb2
                    D2c = dp.tile([128, rb2 * 256], cdt, tag="D2c", name="D2c")
                    D2t = dp.tile([128, rb2 * 256], cdt, tag="D2t", name="D2t")
                    nc.scalar.memzero(D2c[:])
                    nc.scalar.memzero(D2t[:])
                    if cdt == F32:
                        tgc, tgt_ = D2c, D2t
                    else:
                        tgc = dp.tile([128, rb2 * 256], F32, tag="D2cs", name="D2cs")
                        tgt_ = dp.tile([128, rb2 * 256], F32, tag="D2ts", name="D2ts")
                        nc.scalar.memzero(tgc[:])
                        nc.scalar.memzero(tgt_[:])
                    for q in range(rb2):
                        r = r0 + q
                        sl = bass.ts(q, 256)
                        nc.sync.dma_start(
                            tgc[0:56, sl],
                            scratch[r, 0:14336].rearrange("(a b) -> a b", a=56))
                        nc.sync.dma_start(
                            tgc[56:57, q * 256:q * 256 + 1],
                            scratch[r, 14336:14337].rearrange("(a b) -> a b", a=1))
                        nc.sync.dma_start(
                            tgt_[0:56, sl],
                            target[r, 0:14336].rearrange("(a b) -> a b", a=56))
                        nc.sync.dma_start(
                            tgt_[56:57, q * 256:q * 256 + 1],
                            target[r, 14336:14337].rearrange("(a b) -> a b", a=1))
                    if cdt != F32:
                        nc.scalar.copy(D2c[0:57, :], tgc[0:57, :])
                        nc.scalar.copy(D2t[0:57, :], tgt_[0:57, :])

                    B2 = {}
                    for c in range(2):
                        for inp, D in (("c", D2c), ("t", D2t)):
                            br = dp.tile([128, rb2 * 128], cdt, tag=f"B2r{c}{inp}", name=f"B2r{c}{inp}")
                            bi = dp.tile([128, rb2 * 128], cdt, tag=f"B2i{c}{inp}", name=f"B2i{c}{inp}")
                            pa = pp.tile([128, rb2 * 128], F32, tag="st1", name="st1")
                            pai = pp.tile([128, rb2 * 128], F32, tag="st1", name="st1")
                            for q in range(rb2):
                                dsl = slice(q * 256 + c * 128,
                                            q * 256 + c * 128 + 128)
                                osl = bass.ts(q, 128)
                                nc.tensor.matmul(pa[:, osl], lhsT=D[:, dsl],
                                                 rhs=cs["W1r"][:],
                                                 start=True, stop=True)
                                nc.tensor.matmul(pai[:, osl], lhsT=D[:, dsl],
                                                 rhs=cs["W1i"][:],
                                                 start=True, stop=True)
                            _cmul_psum(nc, dp, "tw", br[:], bi[:], pa[:], pai[:],
                                       _b3(cs[f"twBr{c}"][:], rb2, 128),
                                       _b3(cs[f"twBi{c}"][:], rb2, 128), 128, rb2)
                            B2[(c, inp)] = (br, bi)

                    Z2 = {}
                    for inp in ("c", "t"):
                        for f2c in range(2):
                            zr = dp.tile([128, rb2 * 128], cdt, tag=f"Z2r{inp}{f2c}", name=f"Z2r{inp}{f2c}")
                            zi = dp.tile([128, rb2 * 128], cdt, tag=f"Z2i{inp}{f2c}", name=f"Z2i{inp}{f2c}")
                            pzr = pp.tile([128, rb2 * 128], F32, tag="st2", name="st2")
                            pzi = pp.tile([128, rb2 * 128], F32, tag="st2", name="st2")
                            for t2c in range(2):
                                br, bi = B2[(t2c, inp)]
                                nc.tensor.matmul(pzr[:], lhsT=cs[f"W2Br{t2c}{f2c}"][:],
                                                 rhs=br[:], start=(t2c == 0), stop=False)
                                nc.tensor.matmul(pzr[:], lhsT=cs[f"nW2Bi{t2c}{f2c}"][:],
                                                 rhs=bi[:], start=False, stop=(t2c == 1))
                                nc.tensor.matmul(pzi[:], lhsT=cs[f"W2Bi{t2c}{f2c}"][:],
                                                 rhs=br[:], start=(t2c == 0), stop=False)
                                nc.tensor.matmul(pzi[:], lhsT=cs[f"W2Br{t2c}{f2c}"][:],
                                                 rhs=bi[:], start=False, stop=(t2c == 1))
                            nc.scalar.copy(zr[:], pzr[:])
                            nc.scalar.copy(zi[:], pzi[:])
                            Z2[(inp, f2c)] = (zr, zi)

                    S2 = {}
                    tmpc = dp.tile([128, rb2 * 128], F32, tag="tmpc", name="tmpc")
                    for f2c in range(2):
                        zcr, zci = Z2[("c", f2c)]
                        ztr, zti = Z2[("t", f2c)]
                        sr = dp.tile([128, rb2 * 128], cdt, tag=f"S2r{f2c}", name=f"S2r{f2c}")
                        si = dp.tile([128, rb2 * 128], cdt, tag=f"S2i{f2c}", name=f"S2i{f2c}")
                        nc.vector.tensor_tensor(sr[:], zcr[:], ztr[:], op=AT.mult)
                        nc.vector.tensor_tensor(tmpc[:], zci[:], zti[:], op=AT.mult)
                        nc.vector.tensor_tensor(sr[:], sr[:], tmpc[:], op=AT.add)
                        nc.vector.tensor_tensor(si[:], zci[:], ztr[:], op=AT.mult)
                        nc.vector.tensor_tensor(tmpc[:], zcr[:], zti[:], op=AT.mult)
                        nc.vector.tensor_tensor(si[:], si[:], tmpc[:], op=AT.subtract)
                        S2[f2c] = (sr, si)

                    H2r = dp.tile([128, rb2 * 256], cdt, tag="H2r", name="H2r")
                    H2i = dp.tile([128, rb2 * 256], cdt, tag="H2i", name="H2i")
                    for g in range(rb2 // 2):
                        pgr = pp.tile([128, 512], F32, tag="inv", name="inv")
                        pgi = pp.tile([128, 512], F32, tag="inv", name="inv")
                        for q in range(2):
                            qq = g * 2 + q
                            sl, osl = bass.ts(qq, 128), bass.ts(q, 256)
                            for f2c in range(2):
                                sr, si = S2[f2c]
                                nc.tensor.matmul(pgr[:, osl], lhsT=sr[:, sl],
                                                 rhs=cs[f"V2Br{f2c}"][:],
                                                 start=(f2c == 0), stop=False)
                                nc.tensor.matmul(pgr[:, osl], lhsT=si[:, sl],
                                                 rhs=cs[f"nV2Bi{f2c}"][:],
                                                 start=False, stop=(f2c == 1))
                                nc.tensor.matmul(pgi[:, osl], lhsT=sr[:, sl],
                                                 rhs=cs[f"V2Bi{f2c}"][:],
                                                 start=(f2c == 0), stop=False)
                                nc.tensor.matmul(pgi[:, osl], lhsT=si[:, sl],
                                                 rhs=cs[f"V2Br{f2c}"][:],
                                                 start=False, stop=(f2c == 1))
                        gsl = bass.ts(g, 512)
                        _cmul_psum(nc, dp, "tw", H2r[:, gsl], H2i[:, gsl],
                                   pgr[:], pgi[:],
                                   _b3(cs["itwBr"][:], 2, 256),
                                   _b3(cs["itwBi"][:], 2, 256), 256, 2)

                    for g in range(rb2 // 2):
                        gsl = bass.ts(g, 512)
                        pcc = pp.tile([128, 512], F32, tag="cc", name="cc")
                        nc.tensor.matmul(pcc[:], lhsT=cs["V1Br"][:],
                                         rhs=H2r[:, gsl], start=True, stop=False)
                        nc.tensor.matmul(pcc[:], lhsT=cs["nV1Bi"][:],
                                         rhs=H2i[:, gsl], start=False, stop=True)
                        csl = slice((r0 + g * 2) * 256, (r0 + g * 2 + 2) * 256)
                        ccv = ccm_all[:, csl].rearrange("p (a b) -> p a b", b=256)
                        nc.vector.scalar_tensor_tensor(
                            ccv, pcc[:].rearrange("p (a b) -> p a b", b=256),
                            1.0, _b3(cs["maskB"][:], 2, 256),
                            op0=AT.bypass, op1=AT.add)
                        nc.vector.tensor_reduce(
                            allmax[:, r0 + g * 2:r0 + g * 2 + 2], ccv,
                            axis=AX.X, op=AT.max)

            # ---------------- D) argmax -> shifts ----------------
            with tc.tile_pool(name="amax", bufs=1) as dp:
                pt = pp.tile([rows, 128], F32, tag="st1", name="st1")
                nc.tensor.transpose(pt[:], allmax[:, 0:rows], cs["ident"][:])
                tmax = dp.tile([rows, 128], F32, tag="tmax", name="tmax")
                nc.scalar.copy(tmax[:], pt[:])
                rowmax = dp.tile([rows, 1], F32, tag="rowmax", name="rowmax")
                nc.vector.tensor_reduce(rowmax[:], tmax[:], axis=AX.X, op=AT.max)
                prm = pp.tile([1, rows], F32, tag="st2", name="st2")
                nc.tensor.transpose(prm[:], rowmax[:], cs["ident"][0:rows, 0:rows])
                rmT = dp.tile([1, rows], F32, tag="rmT", name="rmT")
                nc.scalar.copy(rmT[:], prm[:])
                pmb = pp.tile([128, rows], F32, tag="inv", name="inv")
                nc.tensor.matmul(pmb[:], lhsT=cs["ones1x128"][:], rhs=rmT[:],
                                 start=True, stop=True)
                Mb = dp.tile([128, rows], F32, tag="Mb", name="Mb")
                nc.scalar.copy(Mb[:], pmb[:])

                eqm = dp.tile([128, min(rows, 8) * 256], BF16, tag="eqm", name="eqm")
                selm = dp.tile([128, min(rows, 8) * 256], F32, tag="selm", name="selm")
                for bb in range(max(1, rows // 8)):
                    csl = bass.ts(bb, min(rows, 8) * 256)
                    nr8 = min(rows, 8)
                    mbb = Mb[:, bb * nr8:(bb + 1) * nr8]\
                        .rearrange("p (a b) -> p a b", b=1)\
                        .to_broadcast([128, nr8, 256])
                    ccv = ccm_all[:, csl].rearrange("p (a b) -> p a b", b=256)
                    nc.vector.tensor_tensor(
                        eqm[:].rearrange("p (a b) -> p a b", b=256),
                        ccv, mbb, op=AT.is_equal)
                    nc.vector.tensor_tensor(
                        selm[:].rearrange("p (a b) -> p a b", b=256),
                        eqm[:].rearrange("p (a b) -> p a b", b=256),
                        _b3(cs["shvB"][:], nr8, 256), op=AT.mult)
                    nc.vector.tensor_reduce(
                        allmin[:, bb * nr8:(bb + 1) * nr8],
                        selm[:].rearrange("p (a b) -> p a b", b=256),
                        axis=AX.X, op=AT.min)
                pt2 = pp.tile([rows, 128], F32, tag="cc", name="cc")
                nc.tensor.transpose(pt2[:], allmin[:, 0:rows], cs["ident"][:])
                tmin = dp.tile([rows, 128], F32, tag="tmin", name="tmin")
                nc.scalar.copy(tmin[:], pt2[:])
                nc.vector.tensor_reduce(shifts[:], tmin[:], axis=AX.X, op=AT.min)
                nc.vector.tensor_scalar_add(shifts[:], shifts[:], BIGL + float(START0))

                # start = (7040 + shift) mod 14337
                m1 = dp.tile([rows, 1], F32, tag="m1", name="m1")
                nc.vector.tensor_scalar(out=m1[:], in0=shifts[:], scalar1=0.0,
                                        scalar2=None, op0=AT.is_lt)
                nc.vector.scalar_tensor_tensor(
                    shifts[:], m1[:], float(CONV_LEN), shifts[:],
                    op0=AT.mult, op1=AT.add)
                nc.vector.tensor_scalar(out=m1[:], in0=shifts[:],
                                        scalar1=float(CONV_LEN), scalar2=None,
                                        op0=AT.is_ge)
                nc.vector.scalar_tensor_tensor(
                    shifts[:], m1[:], float(-CONV_LEN), shifts[:],
                    op0=AT.mult, op1=AT.add)

                idxf = dp.tile([rows, CROP], F32, tag="idxf", name="idxf")
                nc.vector.tensor_tensor(idxf[:], cs["winidx"][0:rows, :],
                                        shifts[:].to_broadcast([rows, CROP]),
                                        op=AT.add)
                idxi = dp.tile([rows, CROP], I32, tag="idxi", name="idxi")
                nc.vector.tensor_copy(idxi[:], idxf[:])
                w = dp.tile([rows, CROP], F32, tag="wg", name="wg")
                nc.gpsimd.indirect_dma_start(
                    out=w[:], out_offset=None,
                    in_=scratch.ap().rearrange("r p -> (r p)").rearrange(
                        "(a b) -> a b", b=1),
                    in_offset=bass.IndirectOffsetOnAxis(ap=idxi[:], axis=0),
                )
                tw_ = dp.tile([rows, CROP], F32, tag="twin", name="twin")
                nc.sync.dma_start(tw_[:], target[:, START0:START0 + CROP])
                nc.vector.tensor_tensor(w[:], w[:], tw_[:], op=AT.subtract)
                convacc = dp.tile([rows, 1], F32, tag="convacc", name="convacc")
                nc.vector.scalar_tensor_tensor(
                    tw_[:], w[:], 1.0, w[:], op0=AT.bypass, op1=AT.mult,
                    accum_out=convacc[:])

                a0 = dp.tile([128, 1], F32, tag="a0", name="a0")
                nc.vector.tensor_reduce(a0[:], astf_acc[:], axis=AX.X, op=AT.add)
                psa = pp.tile([1, 1], F32, tag="st1", name="st1")
                nc.tensor.matmul(psa[:], lhsT=a0[:], rhs=cs["ones128"][:],
                                 start=True, stop=True)
                psc = pp.tile([1, 1], F32, tag="st2", name="st2")
                nc.tensor.matmul(psc[:], lhsT=convacc[:], rhs=cs["ones64"][0:rows, :],
                                 start=True, stop=True)
                nc.scalar.copy(outt[:, 0:1], psa[:])
                nc.scalar.copy(outt[:, 1:2], psc[:])
                nc.sync.dma_start(out[:], outt[:])

    nc.finalize()
    return nc, consts


_CACHE = {}


def get_built(cdt=F32):
    key = str(cdt)
    if key not in _CACHE:
        _CACHE[key] = build_nc(cdt=cdt)
    return _CACHE[key]


LAST_RESULT = {}


def kernel(pred_astf, true_astf, egf, target_waveform):
    import os
    from concourse.bass_utils import run_bass_kernel_spmd
    cdt = BF16 if os.environ.get("CONVALIGN_BF16") == "1" else F32
    nc, consts = get_built(cdt)
    if cdt != F32:
        import ml_dtypes
        from kernel import make_consts as _mk  # noqa
        mmnames = _mm_const_names()
        consts = {k: (v.astype(ml_dtypes.bfloat16) if k in mmnames else v)
                  for k, v in consts.items()}
    pred_astf = np.ascontiguousarray(np.asarray(pred_astf, np.float32))
    true_astf = np.ascontiguousarray(np.asarray(true_astf, np.float32))
    egf = np.ascontiguousarray(np.asarray(egf, np.float32))
    target_waveform = np.ascontiguousarray(
        np.asarray(target_waveform, np.float32))
    B = pred_astf.shape[0]
    per = B // NCORES
    in_maps = []
    for i in range(NCORES):
        sl = slice(i * per, (i + 1) * per)
        m = {"pred": pred_astf[sl], "true": true_astf[sl],
             "egf": egf[sl], "target": target_waveform[sl]}
        m.update(consts)
        in_maps.append(m)
    import os
    trace = os.environ.get("CONVALIGN_TRACE") == "1"
    res = run_bass_kernel_spmd(nc, in_maps, core_ids=list(range(NCORES)),
                               trace=trace)
    LAST_RESULT["res"] = res
    sums = np.stack([res.results[i]["out"][0] for i in range(NCORES)])
    loss_astf = np.float32(sums[:, 0].sum() / (B * L1))
    loss_conv = np.float32(sums[:, 1].sum() / (B * CROP))
    total = np.float32(loss_astf + loss_conv)
    return total, loss_astf, loss_conv



# revision 2
# speedup vs baseline: 1.3324x; 1.3324x over previous
"""Trainium2 Bass kernel for nn_ConvAlignLoss (8-core data parallel).

Self-contained: hardcodes shapes; imports concourse from /opt/trn_rl_repo.

Per core (R=64 rows):
  loss_astf partial: sum((pred-true)^2)
  conv = irfft16384(fft(pred) * conj(fft(egf_pad)))[:14337]  (2-stage matmul FFT)
  cc   = irfft32768(fft(conv_pad) * conj(fft(target_pad)))
  shift = mapped masked argmax of cc (== reference argmax over n=28673)
  loss_conv partial: sum((conv[(7040+i+shift) % 14337] - target[7040+i])^2)
Host combines the 8 cores' (sum_astf, sum_conv) into the scalar losses.

FFT structure (N = 128*N2):
  FWD:  D[t1,t2]=x[N2*t1+t2]; A[t2,f1]=sum_t1 D*W1 (data-stationary matmul);
        B=A*tw; Z[f2,f1]=sum_t2 W2[t2,f2]*B.   Z2d[f2,f1] == X[f1+128*f2]
  INV:  G[f1,t2]=sum_f2 S2d[f2,f1]*V2[f2,t2] (S stationary); H=G*itw;
        x2d[t1,t2]=(1/N) Re(sum_f1 V1[f1,t1]*H[f1,t2])
"""
import sys

sys.path.insert(0, "/opt/trn_rl_repo")

import numpy as np
import concourse.bass as bass
import concourse.bacc as bacc
import concourse.mybir as mybir
from concourse import tile

F32 = mybir.dt.float32
BF16 = mybir.dt.bfloat16
I32 = mybir.dt.int32
AT = mybir.AluOpType
AX = mybir.AxisListType

R = 64
NCORES = 8
L1, L2 = 16384, 2048
CONV_LEN = L1 - L2 + 1      # 14337
N_A, N_B = 16384, 32768
GAP_LO, GAP_HI = CONV_LEN, N_B - CONV_LEN + 1   # gap [14337, 18432)
CROP = 256
START0 = (CONV_LEN - CROP) // 2                 # 7040
PITCH = 14720
BIGL = float(2 ** 23)


def _dft(n, sign):
    k = np.arange(n)
    return np.exp(sign * 2j * np.pi * np.outer(k, k) / n)


def make_consts():
    c = {}

    def put(name, arr, dt=np.float32):
        c[name] = np.ascontiguousarray(np.asarray(arr, np.float64)).astype(dt)

    W1 = _dft(128, -1)
    put("W1r", W1.real); put("W1i", W1.imag); put("nW1i", -W1.imag)
    twA = np.exp(-2j * np.pi * np.outer(np.arange(128), np.arange(128)) / N_A)
    put("twAr", twA.real); put("twAi", twA.imag)
    V2A = _dft(128, +1)
    put("V2Ar", V2A.real); put("V2Ai", V2A.imag); put("nV2Ai", -V2A.imag)
    itwA = np.exp(2j * np.pi * np.outer(np.arange(128), np.arange(128)) / N_A)
    put("itwAr", itwA.real); put("itwAi", itwA.imag)
    V1A = _dft(128, +1) / N_A
    put("V1Ar", V1A.real); put("nV1Ai", -V1A.imag)

    W2B = _dft(256, -1)          # [t2, f2]
    for a in range(2):
        for b in range(2):
            blk = W2B[a * 128:(a + 1) * 128, b * 128:(b + 1) * 128]
            put(f"W2Br{a}{b}", blk.real)
            put(f"W2Bi{a}{b}", blk.imag)
            put(f"nW2Bi{a}{b}", -blk.imag)
    twB = np.exp(-2j * np.pi * np.outer(np.arange(256), np.arange(128)) / N_B)
    for a in range(2):
        put(f"twBr{a}", twB.real[a * 128:(a + 1) * 128])
        put(f"twBi{a}", twB.imag[a * 128:(a + 1) * 128])
    V2B = _dft(256, +1)          # [f2, t2]
    for a in range(2):
        blk = V2B[a * 128:(a + 1) * 128, :]
        put(f"V2Br{a}", blk.real)
        put(f"V2Bi{a}", blk.imag)
        put(f"nV2Bi{a}", -blk.imag)
    itwB = np.exp(2j * np.pi * np.outer(np.arange(128), np.arange(256)) / N_B)
    put("itwBr", itwB.real); put("itwBi", itwB.imag)
    V1B = _dft(128, +1) / N_B
    put("V1Br", V1B.real); put("nV1Bi", -V1B.imag)

    put("ident", np.eye(128))
    put("ones1x128", np.ones((1, 128)))
    put("ones128", np.ones((128, 1)))
    put("ones64", np.ones((64, 1)))

    j = np.arange(128)[:, None] * 256 + np.arange(256)[None, :]   # [t1, t2]
    gap = (j >= GAP_LO) & (j < GAP_HI)
    put("maskB", np.where(gap, -1e30, 0.0))
    shiftval = np.where(j <= CONV_LEN - 1, j - (CONV_LEN - 1), j - GAP_HI + 1)
    put("shvB", np.where(gap, 0.0, shiftval - BIGL))
    put("winidx", np.arange(R)[:, None] * PITCH
        + np.arange(CROP)[None, :])                               # [64, 256]
    return c


def _b3(ap, n, inner):
    """[128, inner] const AP -> [128, n, inner] broadcast over middle dim."""
    return ap.rearrange("p (a b) -> p a b", a=1).to_broadcast([128, n, inner])


def _cmul_psum(nc, pool, tag, outr, outi, pr, pi, twr, twi, inner, n):
    """(outr + i outi) = (pr + i pi) * (twr + i twi); p* in PSUM, tw const APs
    broadcast over n blocks of `inner`. outr/outi are SBUF APs [128, n*inner]."""
    tmp = pool.tile([128, n * inner], F32, tag="twtmp", name="twtmp")
    orv = outr.rearrange("p (a b) -> p a b", b=inner)
    oiv = outi.rearrange("p (a b) -> p a b", b=inner)
    prv = pr.rearrange("p (a b) -> p a b", b=inner)
    piv = pi.rearrange("p (a b) -> p a b", b=inner)
    tv = tmp[:].rearrange("p (a b) -> p a b", b=inner)
    nc.vector.tensor_tensor(orv, prv, twr, op=AT.mult)
    nc.vector.tensor_tensor(tv, piv, twi, op=AT.mult)
    nc.vector.tensor_tensor(orv, orv, tv, op=AT.subtract)
    nc.vector.tensor_tensor(oiv, prv, twi, op=AT.mult)
    nc.vector.tensor_tensor(tv, piv, twr, op=AT.mult)
    nc.vector.tensor_tensor(oiv, oiv, tv, op=AT.add)


def _mm_const_names():
    s = {"W1r", "W1i", "nW1i", "V2Ar", "V2Ai", "nV2Ai", "V1Ar", "nV1Ai",
         "itwAr", "itwAi", "twAr", "twAi", "itwBr", "itwBi", "V1Br",
         "nV1Bi"}
    s |= {f"W2Br{a}{b}" for a in range(2) for b in range(2)}
    s |= {f"W2Bi{a}{b}" for a in range(2) for b in range(2)}
    s |= {f"nW2Bi{a}{b}" for a in range(2) for b in range(2)}
    s |= {f"twBr{a}" for a in range(2)} | {f"twBi{a}" for a in range(2)}
    s |= {f"V2Br{a}" for a in range(2)} | {f"V2Bi{a}" for a in range(2)}
    s |= {f"nV2Bi{a}" for a in range(2)}
    return s


def build_nc(cdt=F32, rows=R, rbb=8, rb2=4):
    nc = bacc.Bacc("TRN2", target_bir_lowering=False, debug=False,
                   num_devices=NCORES)
    consts = make_consts()

    pred = nc.dram_tensor("pred", [rows, L1], F32, kind="ExternalInput")
    true_ = nc.dram_tensor("true", [rows, L1], F32, kind="ExternalInput")
    egf = nc.dram_tensor("egf", [rows, L2], F32, kind="ExternalInput")
    target = nc.dram_tensor("target", [rows, CONV_LEN], F32,
                            kind="ExternalInput")
    out = nc.dram_tensor("out", [1, 2], F32, kind="ExternalOutput")
    scratch = nc.dram_tensor("scratch", [rows, PITCH], F32)

    MM_CONST = _mm_const_names()

    cdram = {}
    for name, arr in consts.items():
        cdt_n = cdt if name in MM_CONST else F32
        cdram[name] = nc.dram_tensor(name, list(arr.shape), cdt_n,
                                     kind="ExternalInput")

    nb1, nb2 = rows // rbb, rows // rb2

    with tile.TileContext(nc) as tc:
        with (
            tc.tile_pool(name="consts", bufs=1) as cpool,
            tc.tile_pool(name="keep", bufs=1) as kpool,
            tc.tile_pool(name="ps", bufs=2, space="PSUM") as pp,
        ):
            cs = {}
            for name, arr in consts.items():
                dt = cdt if name in MM_CONST else F32
                t = cpool.tile(list(arr.shape), dt, tag=f"c_{name}", name=f"c_{name}")
                nc.sync.dma_start(t[:], cdram[name][:])
                cs[name] = t

            allmax = kpool.tile([128, rows], F32, tag="allmax", name="allmax")
            allmin = kpool.tile([128, rows], F32, tag="allmin", name="allmin")
            ccm_all = kpool.tile([128, rows * 256], BF16, tag="ccm", name="ccm")
            astf_acc = kpool.tile([128, 8], F32, tag="astfacc", name="astfacc")
            shifts = kpool.tile([rows, 1], F32, tag="shifts", name="shifts")
            outt = kpool.tile([1, 2], F32, tag="outt", name="outt")

            # ---------------- A) astf ----------------
            predf = pred.ap().rearrange("r l -> (r l)").rearrange(
                "(p f) -> p f", p=128)
            truef = true_.ap().rearrange("r l -> (r l)").rearrange(
                "(p f) -> p f", p=128)
            fch = rows * L1 // 128 // 8
            with tc.tile_pool(name="astf", bufs=2) as apool:
                for i in range(8):
                    tp = apool.tile([128, fch], F32, tag="ap", name="ap")
                    tt = apool.tile([128, fch], F32, tag="at", name="at")
                    sl = bass.ts(i, fch)
                    nc.sync.dma_start(tp[:], predf[:, sl])
                    nc.sync.dma_start(tt[:], truef[:, sl])
                    nc.vector.tensor_tensor(tt[:], tp[:], tt[:], op=AT.subtract)
                    nc.vector.scalar_tensor_tensor(
                        tp[:], tt[:], 1.0, tt[:], op0=AT.bypass, op1=AT.mult,
                        accum_out=astf_acc[:, i:i + 1])

            # ---------------- B) 16K level ----------------
            with tc.tile_pool(name="p16", bufs=1) as dp:
                for b in range(nb1):
                    r0 = b * rbb
                    Dp = dp.tile([128, rbb * 128], cdt, tag="Dp", name="Dp")
                    De = dp.tile([128, rbb * 128], cdt, tag="De", name="De")
                    nc.scalar.memzero(De[:])
                    if cdt == F32:
                        for q in range(rbb):
                            r = r0 + q
                            nc.sync.dma_start(
                                Dp[:, bass.ts(q, 128)],
                                pred[r, :].rearrange("(a b) -> a b", a=128))
                            nc.sync.dma_start(
                                De[:16, bass.ts(q, 128)],
                                egf[r, :].rearrange("(a b) -> a b", a=16))
                    else:
                        Dst = dp.tile([128, rbb * 128], F32, tag="Dst", name="Dst")
                        Est = dp.tile([16, rbb * 128], F32, tag="Est", name="Est")
                        for q in range(rbb):
                            r = r0 + q
                            nc.sync.dma_start(
                                Dst[:, bass.ts(q, 128)],
                                pred[r, :].rearrange("(a b) -> a b", a=128))
                            nc.sync.dma_start(
                                Est[:16, bass.ts(q, 128)],
                                egf[r, :].rearrange("(a b) -> a b", a=16))
                        nc.scalar.copy(Dp[:], Dst[:])
                        nc.scalar.copy(De[:16, :], Est[:16, :])

                    Bs = {k: dp.tile([128, rbb * 128], cdt, tag=f"B{k}", name=f"B{k}")
                          for k in ("pr", "pi", "er", "ei")}
                    for g in range(rbb // 4):
                        gsl = bass.ts(g, 512)
                        for inp, D in (("p", Dp), ("e", De)):
                            pa = pp.tile([128, 512], F32, tag="st1", name="st1")
                            pai = pp.tile([128, 512], F32, tag="st1", name="st1")
                            for q in range(4):
                                qq = g * 4 + q
                                sl, osl = bass.ts(qq, 128), bass.ts(q, 128)
                                nc.tensor.matmul(pa[:, osl], lhsT=D[:, sl],
                                                 rhs=cs["W1r"][:],
                                                 start=True, stop=True)
                                nc.tensor.matmul(pai[:, osl], lhsT=D[:, sl],
                                                 rhs=cs["W1i"][:],
                                                 start=True, stop=True)
                            _cmul_psum(nc, dp, "tw",
                                       Bs[inp + "r"][:, gsl], Bs[inp + "i"][:, gsl],
                                       pa[:], pai[:],
                                       _b3(cs["twAr"][:], 4, 128),
                                       _b3(cs["twAi"][:], 4, 128), 128, 4)

                    Zs = {k: dp.tile([128, rbb * 128], cdt, tag=f"Z{k}", name=f"Z{k}")
                          for k in ("pr", "pi", "er", "ei")}
                    for g in range(rbb // 4):
                        gsl = bass.ts(g, 512)
                        for inp in ("p", "e"):
                            br, bi = Bs[inp + "r"], Bs[inp + "i"]
                            pzr = pp.tile([128, 512], F32, tag="st2", name="st2")
                            pzi = pp.tile([128, 512], F32, tag="st2", name="st2")
                            nc.tensor.matmul(pzr[:], lhsT=cs["W1r"][:],
                                             rhs=br[:, gsl], start=True, stop=False)
                            nc.tensor.matmul(pzr[:], lhsT=cs["nW1i"][:],
                                             rhs=bi[:, gsl], start=False, stop=True)
                            nc.tensor.matmul(pzi[:], lhsT=cs["W1i"][:],
                                             rhs=br[:, gsl], start=True, stop=False)
                            nc.tensor.matmul(pzi[:], lhsT=cs["W1r"][:],
                                             rhs=bi[:, gsl], start=False, stop=True)
                            nc.scalar.copy(Zs[inp + "r"][:, gsl], pzr[:])
                            nc.scalar.copy(Zs[inp + "i"][:, gsl], pzi[:])

                    Sr = dp.tile([128, rbb * 128], cdt, tag="Sr", name="Sr")
                    Si = dp.tile([128, rbb * 128], cdt, tag="Si", name="Si")
                    tmpb = dp.tile([128, rbb * 128], F32, tag="tmpbig", name="tmpbig")
                    nc.vector.tensor_tensor(Sr[:], Zs["pr"][:], Zs["er"][:], op=AT.mult)
                    nc.vector.tensor_tensor(tmpb[:], Zs["pi"][:], Zs["ei"][:], op=AT.mult)
                    nc.vector.tensor_tensor(Sr[:], Sr[:], tmpb[:], op=AT.add)
                    nc.vector.tensor_tensor(Si[:], Zs["pi"][:], Zs["er"][:], op=AT.mult)
                    nc.vector.tensor_tensor(tmpb[:], Zs["pr"][:], Zs["ei"][:], op=AT.mult)
                    nc.vector.tensor_tensor(Si[:], Si[:], tmpb[:], op=AT.subtract)

                    Hr = dp.tile([128, rbb * 128], cdt, tag="Hr", name="Hr")
                    Hi = dp.tile([128, rbb * 128], cdt, tag="Hi", name="Hi")
                    for g in range(rbb // 4):
                        gsl = bass.ts(g, 512)
                        pgr = pp.tile([128, 512], F32, tag="inv", name="inv")
                        pgi = pp.tile([128, 512], F32, tag="inv", name="inv")
                        for q in range(4):
                            qq = g * 4 + q
                            sl, osl = bass.ts(qq, 128), bass.ts(q, 128)
                            nc.tensor.matmul(pgr[:, osl], lhsT=Sr[:, sl],
                                             rhs=cs["V2Ar"][:], start=True, stop=False)
                            nc.tensor.matmul(pgr[:, osl], lhsT=Si[:, sl],
                                             rhs=cs["nV2Ai"][:], start=False, stop=True)
                            nc.tensor.matmul(pgi[:, osl], lhsT=Sr[:, sl],
                                             rhs=cs["V2Ai"][:], start=True, stop=False)
                            nc.tensor.matmul(pgi[:, osl], lhsT=Si[:, sl],
                                             rhs=cs["V2Ar"][:], start=False, stop=True)
                        _cmul_psum(nc, dp, "tw", Hr[:, gsl], Hi[:, gsl],
                                   pgr[:], pgi[:],
                                   _b3(cs["itwAr"][:], 4, 128),
                                   _b3(cs["itwAi"][:], 4, 128), 128, 4)

                    convSB = dp.tile([128, rbb * 128], F32, tag="convSB", name="convSB")
                    for g in range(rbb // 4):
                        gsl = bass.ts(g, 512)
                        pc = pp.tile([128, 512], F32, tag="cc", name="cc")
                        nc.tensor.matmul(pc[:], lhsT=cs["V1Ar"][:],
                                         rhs=Hr[:, gsl], start=True, stop=False)
                        nc.tensor.matmul(pc[:], lhsT=cs["nV1Ai"][:],
                                         rhs=Hi[:, gsl], start=False, stop=True)
                        nc.scalar.copy(convSB[:, gsl], pc[:])

                    for q in range(rbb):
                        r = r0 + q
                        csl = bass.ts(q, 128)
                        nc.sync.dma_start(
                            scratch[r, 0:14336].rearrange("(a b) -> a b", a=112),
                            convSB[0:112, csl])
                        nc.sync.dma_start(
                            scratch[r, 14336:14337].rearrange("(a b) -> a b", a=1),
                            convSB[112:113, q * 128:q * 128 + 1])
                        nc.sync.dma_start(
                            scratch[r, 14337:14593].rearrange("(a b) -> a b", a=2),
                            convSB[0:2, csl])

            # ---------------- C) 32K level ----------------
            with tc.tile_pool(name="p32", bufs=1) as dp:
                for b in range(nb2):
                    r0 = b * rb2
                    D2c = dp.tile([128, rb2 * 256], cdt, tag="D2c", name="D2c")
                    D2t = dp.tile([128, rb2 * 256], cdt, tag="D2t", name="D2t")
                    nc.scalar.memzero(D2c[:])
                    nc.scalar.memzero(D2t[:])
                    if cdt == F32:
                        tgc, tgt_ = D2c, D2t
                    else:
                        tgc = dp.tile([128, rb2 * 256], F32, tag="D2cs", name="D2cs")
                        tgt_ = dp.tile([128, rb2 * 256], F32, tag="D2ts", name="D2ts")
                        nc.scalar.memzero(tgc[:])
                        nc.scalar.memzero(tgt_[:])
                    for q in range(rb2):
                        r = r0 + q
                        sl = bass.ts(q, 256)
                        nc.sync.dma_start(
                            tgc[0:56, sl],
                            scratch[r, 0:14336].rearrange("(a b) -> a b", a=56))
                        nc.sync.dma_start(
                            tgc[56:57, q * 256:q * 256 + 1],
                            scratch[r, 14336:14337].rearrange("(a b) -> a b", a=1))
                        nc.sync.dma_start(
                            tgt_[0:56, sl],
                            target[r, 0:14336].rearrange("(a b) -> a b", a=56))
                        nc.sync.dma_start(
                            tgt_[56:57, q * 256:q * 256 + 1],
                            target[r, 14336:14337].rearrange("(a b) -> a b", a=1))
                    if cdt != F32:
                        nc.scalar.copy(D2c[0:57, :], tgc[0:57, :])
                        nc.scalar.copy(D2t[0:57, :], tgt_[0:57, :])

                    B2 = {}
                    for c in range(2):
                        for inp, D in (("c", D2c), ("t", D2t)):
                            br = dp.tile([128, rb2 * 128], cdt, tag=f"B2r{c}{inp}", name=f"B2r{c}{inp}")
                            bi = dp.tile([128, rb2 * 128], cdt, tag=f"B2i{c}{inp}", name=f"B2i{c}{inp}")
                            pa = pp.tile([128, rb2 * 128], F32, tag="st1", name="st1")
                            pai = pp.tile([128, rb2 * 128], F32, tag="st1", name="st1")
                            for q in range(rb2):
                                dsl = slice(q * 256 + c * 128,
                                            q * 256 + c * 128 + 128)
                                osl = bass.ts(q, 128)
                                nc.tensor.matmul(pa[:, osl], lhsT=D[:, dsl],
                                                 rhs=cs["W1r"][:],
                                                 start=True, stop=True)
                                nc.tensor.matmul(pai[:, osl], lhsT=D[:, dsl],
                                                 rhs=cs["W1i"][:],
                                                 start=True, stop=True)
                            _cmul_psum(nc, dp, "tw", br[:], bi[:], pa[:], pai[:],
                                       _b3(cs[f"twBr{c}"][:], rb2, 128),
                                       _b3(cs[f"twBi{c}"][:], rb2, 128), 128, rb2)
                            B2[(c, inp)] = (br, bi)

                    Z2 = {}
                    for inp in ("c", "t"):
                        for f2c in range(2):
                            zr = dp.tile([128, rb2 * 128], cdt, tag=f"Z2r{inp}{f2c}", name=f"Z2r{inp}{f2c}")
                            zi = dp.tile([128, rb2 * 128], cdt, tag=f"Z2i{inp}{f2c}", name=f"Z2i{inp}{f2c}")
                            pzr = pp.tile([128, rb2 * 128], F32, tag="st2", name="st2")
                            pzi = pp.tile([128, rb2 * 128], F32, tag="st2", name="st2")
                            for t2c in range(2):
                                br, bi = B2[(t2c, inp)]
                                nc.tensor.matmul(pzr[:], lhsT=cs[f"W2Br{t2c}{f2c}"][:],
                                                 rhs=br[:], start=(t2c == 0), stop=False)
                                nc.tensor.matmul(pzr[:], lhsT=cs[f"nW2Bi{t2c}{f2c}"][:],
                                                 rhs=bi[:], start=False, stop=(t2c == 1))
                                nc.tensor.matmul(pzi[:], lhsT=cs[f"W2Bi{t2c}{f2c}"][:],
                                                 rhs=br[:], start=(t2c == 0), stop=False)
                                nc.tensor.matmul(pzi[:], lhsT=cs[f"W2Br{t2c}{f2c}"][:],
                                                 rhs=bi[:], start=False, stop=(t2c == 1))
                            nc.scalar.copy(zr[:], pzr[:])
                            nc.scalar.copy(zi[:], pzi[:])
                            Z2[(inp, f2c)] = (zr, zi)

                    S2 = {}
                    tmpc = dp.tile([128, rb2 * 128], F32, tag="tmpc", name="tmpc")
                    for f2c in range(2):
                        zcr, zci = Z2[("c", f2c)]
                        ztr, zti = Z2[("t", f2c)]
                        sr = dp.tile([128, rb2 * 128], cdt, tag=f"S2r{f2c}", name=f"S2r{f2c}")
                        si = dp.tile([128, rb2 * 128], cdt, tag=f"S2i{f2c}", name=f"S2i{f2c}")
                        nc.vector.tensor_tensor(sr[:], zcr[:], ztr[:], op=AT.mult)
                        nc.vector.tensor_tensor(tmpc[:], zci[:], zti[:], op=AT.mult)
                        nc.vector.tensor_tensor(sr[:], sr[:], tmpc[:], op=AT.add)
                        nc.vector.tensor_tensor(si[:], zci[:], ztr[:], op=AT.mult)
                        nc.vector.tensor_tensor(tmpc[:], zcr[:], zti[:], op=AT.mult)
                        nc.vector.tensor_tensor(si[:], si[:], tmpc[:], op=AT.subtract)
                        S2[f2c] = (sr, si)

                    H2r = dp.tile([128, rb2 * 256], cdt, tag="H2r", name="H2r")
                    H2i = dp.tile([128, rb2 * 256], cdt, tag="H2i", name="H2i")
                    for g in range(rb2 // 2):
                        pgr = pp.tile([128, 512], F32, tag="inv", name="inv")
                        pgi = pp.tile([128, 512], F32, tag="inv", name="inv")
                        for q in range(2):
                            qq = g * 2 + q
                            sl, osl = bass.ts(qq, 128), bass.ts(q, 256)
                            for f2c in range(2):
                                sr, si = S2[f2c]
                                nc.tensor.matmul(pgr[:, osl], lhsT=sr[:, sl],
                                                 rhs=cs[f"V2Br{f2c}"][:],
                                                 start=(f2c == 0), stop=False)
                                nc.tensor.matmul(pgr[:, osl], lhsT=si[:, sl],
                                                 rhs=cs[f"nV2Bi{f2c}"][:],
                                                 start=False, stop=(f2c == 1))
                                nc.tensor.matmul(pgi[:, osl], lhsT=sr[:, sl],
                                                 rhs=cs[f"V2Bi{f2c}"][:],
                                                 start=(f2c == 0), stop=False)
                                nc.tensor.matmul(pgi[:, osl], lhsT=si[:, sl],
                                                 rhs=cs[f"V2Br{f2c}"][:],
                                                 start=False, stop=(f2c == 1))
                        gsl = bass.ts(g, 512)
                        _cmul_psum(nc, dp, "tw", H2r[:, gsl], H2i[:, gsl],
                                   pgr[:], pgi[:],
                                   _b3(cs["itwBr"][:], 2, 256),
                                   _b3(cs["itwBi"][:], 2, 256), 256, 2)

                    for g in range(rb2 // 2):
                        gsl = bass.ts(g, 512)
                        pcc = pp.tile([128, 512], F32, tag="cc", name="cc")
                        nc.tensor.matmul(pcc[:], lhsT=cs["V1Br"][:],
                                         rhs=H2r[:, gsl], start=True, stop=False)
                        nc.tensor.matmul(pcc[:], lhsT=cs["nV1Bi"][:],
                                         rhs=H2i[:, gsl], start=False, stop=True)
                        csl = slice((r0 + g * 2) * 256, (r0 + g * 2 + 2) * 256)
                        ccv = ccm_all[:, csl].rearrange("p (a b) -> p a b", b=256)
                        nc.vector.scalar_tensor_tensor(
                            ccv, pcc[:].rearrange("p (a b) -> p a b", b=256),
                            1.0, _b3(cs["maskB"][:], 2, 256),
                            op0=AT.bypass, op1=AT.add)
                        nc.vector.tensor_reduce(
                            allmax[:, r0 + g * 2:r0 + g * 2 + 2], ccv,
                            axis=AX.X, op=AT.max)

            # ---------------- D) argmax -> shifts ----------------
            with tc.tile_pool(name="amax", bufs=1) as dp:
                pt = pp.tile([rows, 128], F32, tag="st1", name="st1")
                nc.tensor.transpose(pt[:], allmax[:, 0:rows], cs["ident"][:])
                tmax = dp.tile([rows, 128], F32, tag="tmax", name="tmax")
                nc.scalar.copy(tmax[:], pt[:])
                rowmax = dp.tile([rows, 1], F32, tag="rowmax", name="rowmax")
                nc.vector.tensor_reduce(rowmax[:], tmax[:], axis=AX.X, op=AT.max)
                prm = pp.tile([1, rows], F32, tag="st2", name="st2")
                nc.tensor.transpose(prm[:], rowmax[:], cs["ident"][0:rows, 0:rows])
                rmT = dp.tile([1, rows], F32, tag="rmT", name="rmT")
                nc.scalar.copy(rmT[:], prm[:])
                pmb = pp.tile([128, rows], F32, tag="inv", name="inv")
                nc.tensor.matmul(pmb[:], lhsT=cs["ones1x128"][:], rhs=rmT[:],
                                 start=True, stop=True)
                Mb = dp.tile([128, rows], F32, tag="Mb", name="Mb")
                nc.scalar.copy(Mb[:], pmb[:])

                eqm = dp.tile([128, min(rows, 8) * 256], BF16, tag="eqm", name="eqm")
                selm = dp.tile([128, min(rows, 8) * 256], F32, tag="selm", name="selm")
                for bb in range(max(1, rows // 8)):
                    csl = bass.ts(bb, min(rows, 8) * 256)
                    nr8 = min(rows, 8)
                    mbb = Mb[:, bb * nr8:(bb + 1) * nr8]\
                        .rearrange("p (a b) -> p a b", b=1)\
                        .to_broadcast([128, nr8, 256])
                    ccv = ccm_all[:, csl].rearrange("p (a b) -> p a b", b=256)
                    nc.vector.tensor_tensor(
                        eqm[:].rearrange("p (a b) -> p a b", b=256),
                        ccv, mbb, op=AT.is_equal)
                    nc.vector.tensor_tensor(
                        selm[:].rearrange("p (a b) -> p a b", b=256),
                        eqm[:].rearrange("p (a b) -> p a b", b=256),
                        _b3(cs["shvB"][:], nr8, 256), op=AT.mult)
                    nc.vector.tensor_reduce(
                        allmin[:, bb * nr8:(bb + 1) * nr8],
                        selm[:].rearrange("p (a b) -> p a b", b=256),
                        axis=AX.X, op=AT.min)
                pt2 = pp.tile([rows, 128], F32, tag="cc", name="cc")
                nc.tensor.transpose(pt2[:], allmin[:, 0:rows], cs["ident"][:])
                tmin = dp.tile([rows, 128], F32, tag="tmin", name="tmin")
                nc.scalar.copy(tmin[:], pt2[:])
                nc.vector.tensor_reduce(shifts[:], tmin[:], axis=AX.X, op=AT.min)
                nc.vector.tensor_scalar_add(shifts[:], shifts[:], BIGL + float(START0))

                # start = (7040 + shift) mod 14337
                m1 = dp.tile([rows, 1], F32, tag="m1", name="m1")
                nc.vector.tensor_scalar(out=m1[:], in0=shifts[:], scalar1=0.0,
                                        scalar2=None, op0=AT.is_lt)
                nc.vector.scalar_tensor_tensor(
                    shifts[:], m1[:], float(CONV_LEN), shifts[:],
                    op0=AT.mult, op1=AT.add)
                nc.vector.tensor_scalar(out=m1[:], in0=shifts[:],
                                        scalar1=float(CONV_LEN), scalar2=None,
                                        op0=AT.is_ge)
                nc.vector.scalar_tensor_tensor(
                    shifts[:], m1[:], float(-CONV_LEN), shifts[:],
                    op0=AT.mult, op1=AT.add)

                idxf = dp.tile([rows, CROP], F32, tag="idxf", name="idxf")
                nc.vector.tensor_tensor(idxf[:], cs["winidx"][0:rows, :],
                                        shifts[:].to_broadcast([rows, CROP]),
                                        op=AT.add)
                idxi = dp.tile([rows, CROP], I32, tag="idxi", name="idxi")
                nc.vector.tensor_copy(idxi[:], idxf[:])
                w = dp.tile([rows, CROP], F32, tag="wg", name="wg")
                nc.gpsimd.indirect_dma_start(
                    out=w[:], out_offset=None,
                    in_=scratch.ap().rearrange("r p -> (r p)").rearrange(
                        "(a b) -> a b", b=1),
                    in_offset=bass.IndirectOffsetOnAxis(ap=idxi[:], axis=0),
                )
                tw_ = dp.tile([rows, CROP], F32, tag="twin", name="twin")
                nc.sync.dma_start(tw_[:], target[:, START0:START0 + CROP])
                nc.vector.tensor_tensor(w[:], w[:], tw_[:], op=AT.subtract)
                convacc = dp.tile([rows, 1], F32, tag="convacc", name="convacc")
                nc.vector.scalar_tensor_tensor(
                    tw_[:], w[:], 1.0, w[:], op0=AT.bypass, op1=AT.mult,
                    accum_out=convacc[:])

                a0 = dp.tile([128, 1], F32, tag="a0", name="a0")
                nc.vector.tensor_reduce(a0[:], astf_acc[:], axis=AX.X, op=AT.add)
                psa = pp.tile([1, 1], F32, tag="st1", name="st1")
                nc.tensor.matmul(psa[:], lhsT=a0[:], rhs=cs["ones128"][:],
                                 start=True, stop=True)
                psc = pp.tile([1, 1], F32, tag="st2", name="st2")
                nc.tensor.matmul(psc[:], lhsT=convacc[:], rhs=cs["ones64"][0:rows, :],
                                 start=True, stop=True)
                nc.scalar.copy(outt[:, 0:1], psa[:])
                nc.scalar.copy(outt[:, 1:2], psc[:])
                nc.sync.dma_start(out[:], outt[:])

    nc.finalize()
    return nc, consts


_CACHE = {}


def get_built(cdt=F32):
    key = str(cdt)
    if key not in _CACHE:
        _CACHE[key] = build_nc(cdt=cdt)
    return _CACHE[key]


LAST_RESULT = {}


def kernel(pred_astf, true_astf, egf, target_waveform):
    import os
    from concourse.bass_utils import run_bass_kernel_spmd
    cdt = BF16 if os.environ.get("CONVALIGN_BF16") == "1" else F32
    nc, consts = get_built(cdt)
    if cdt != F32:
        import ml_dtypes
        from kernel import make_consts as _mk  # noqa
        mmnames = _mm_const_names()
        consts = {k: (v.astype(ml_dtypes.bfloat16) if k in mmnames else v)
                  for k, v in consts.items()}
    pred_astf = np.ascontiguousarray(np.asarray(pred_astf, np.float32))
    true_astf = np.ascontiguousarray(np.asarray(true_astf, np.float32))
    egf = np.ascontiguousarray(np.asarray(egf, np.float32))
    target_waveform = np.ascontiguousarray(
        np.asarray(target_waveform, np.float32))
    B = pred_astf.shape[0]
    per = B // NCORES
    in_maps = []
    for i in range(NCORES):
        sl = slice(i * per, (i + 1) * per)
        m = {"pred": pred_astf[sl], "true": true_astf[sl],
             "egf": egf[sl], "target": target_waveform[sl]}
        m.update(consts)
        in_maps.append(m)
    import os
    trace = os.environ.get("CONVALIGN_TRACE") == "1"
    res = run_bass_kernel_spmd(nc, in_maps, core_ids=list(range(NCORES)),
                               trace=trace)
    LAST_RESULT["res"] = res
    sums = np.stack([res.results[i]["out"][0] for i in range(NCORES)])
    loss_astf = np.float32(sums[:, 0].sum() / (B * L1))
    loss_conv = np.float32(sums[:, 1].sum() / (B * CROP))
    total = np.float32(loss_astf + loss_conv)
    return total, loss_astf, loss_conv



# revision 11
# speedup vs baseline: 1.7286x; 1.2974x over previous
"""Trainium2 Bass kernel for nn_ConvAlignLoss (8-core data parallel).

Self-contained: hardcodes shapes; imports concourse from /opt/trn_rl_repo.

Per core (R=64 rows):
  loss_astf partial: sum((pred-true)^2)
  conv = irfft16384(fft(pred) * conj(fft(egf_pad)))[:14337]  (2-stage matmul FFT)
  cc   = irfft32768(fft(conv_pad) * conj(fft(target_pad)))
  shift = mapped masked argmax of cc (== reference argmax over n=28673)
  loss_conv partial: sum((conv[(7040+i+shift) % 14337] - target[7040+i])^2)
Host combines the 8 cores' (sum_astf, sum_conv) into the scalar losses.

FFT structure (N = 128*N2), Hermitian-cropped for real inputs:
  FWD:  D[t1,t2]=x[N2*t1+t2]; A[t2,f1]=sum_t1 D*W1[:, :65]; B=A*tw;
        Z[f2,f1]=sum_t2 W2[t2,f2]*B  -- only f1 in [0,64] kept.
  INV (Hermitian S):  G[f1,t2]=sum_f2 S2d[f2,f1]*V2[f2,t2]; H=G*itw;
        x2d[t1,t2]=Re(sum_{f1<=64} c_f1 V1[f1,t1]*H[f1,t2])/N,
        c_f1 = 1 for f1 in {0,64}, else 2.
"""
import sys

sys.path.insert(0, "/opt/trn_rl_repo")

import numpy as np
import concourse.bass as bass
import concourse.bacc as bacc
import concourse.mybir as mybir
from concourse import tile

F32 = mybir.dt.float32
BF16 = mybir.dt.bfloat16
I32 = mybir.dt.int32
AT = mybir.AluOpType
AX = mybir.AxisListType

R = 64
NCORES = 8
L1, L2 = 16384, 2048
CONV_LEN = L1 - L2 + 1      # 14337
N_A, N_B = 16384, 32768
GAP_LO, GAP_HI = CONV_LEN, N_B - CONV_LEN + 1   # gap [14337, 18432)
CROP = 256
START0 = (CONV_LEN - CROP) // 2                 # 7040
PITCH = 14720
BIGL = float(2 ** 23)
F1 = 65                      # Hermitian half: f1 in [0, 64]


def _dft(n, sign):
    k = np.arange(n)
    return np.exp(sign * 2j * np.pi * np.outer(k, k) / n)


def make_consts():
    c = {}

    def put(name, arr, dt=np.float32):
        c[name] = np.ascontiguousarray(np.asarray(arr, np.float64)).astype(dt)

    cf = np.ones(F1)
    cf[1:64] = 2.0

    W1 = _dft(128, -1)
    put("W1r", W1.real); put("W1i", W1.imag); put("nW1i", -W1.imag)
    put("W1r65", W1.real[:, :F1]); put("W1i65", W1.imag[:, :F1])
    twA = np.exp(-2j * np.pi * np.outer(np.arange(128), np.arange(F1)) / N_A)
    put("twAr", twA.real); put("twAi", twA.imag)
    V2A = _dft(128, +1)
    put("V2Ar", V2A.real); put("V2Ai", V2A.imag); put("nV2Ai", -V2A.imag)
    itwA = np.exp(2j * np.pi * np.outer(np.arange(F1), np.arange(128)) / N_A)
    put("itwAr", itwA.real); put("itwAi", itwA.imag)
    V1A = cf[:, None] * _dft(128, +1)[:F1] / N_A        # [f1<=64, t1]
    put("V1Ar", V1A.real); put("nV1Ai", -V1A.imag)

    W2B = _dft(256, -1)          # [t2, f2]
    for a in range(2):
        for b in range(2):
            blk = W2B[a * 128:(a + 1) * 128, b * 128:(b + 1) * 128]
            put(f"W2Br{a}{b}", blk.real)
            put(f"W2Bi{a}{b}", blk.imag)
            put(f"nW2Bi{a}{b}", -blk.imag)
    twB = np.exp(-2j * np.pi * np.outer(np.arange(256), np.arange(F1)) / N_B)
    for a in range(2):
        put(f"twBr{a}", twB.real[a * 128:(a + 1) * 128])
        put(f"twBi{a}", twB.imag[a * 128:(a + 1) * 128])
    V2B = _dft(256, +1)          # [f2, t2]
    for a in range(2):
        blk = V2B[a * 128:(a + 1) * 128, :]
        put(f"V2Br{a}", blk.real)
        put(f"V2Bi{a}", blk.imag)
        put(f"nV2Bi{a}", -blk.imag)
    itwB = np.exp(2j * np.pi * np.outer(np.arange(F1), np.arange(256)) / N_B)
    put("itwBr", itwB.real); put("itwBi", itwB.imag)
    V1B = cf[:, None] * _dft(128, +1)[:F1] / N_B        # [f1<=64, t1]
    put("V1Br", V1B.real); put("nV1Bi", -V1B.imag)

    put("ident", np.eye(128))
    put("ones1x128", np.ones((1, 128)))
    put("ones128", np.ones((128, 1)))
    put("ones64", np.ones((64, 1)))

    j = np.arange(128)[:, None] * 256 + np.arange(256)[None, :]   # [t1, t2]
    gap = (j >= GAP_LO) & (j < GAP_HI)
    put("maskB", np.where(gap, -1e30, 0.0))
    shiftval = np.where(j <= CONV_LEN - 1, j - (CONV_LEN - 1), j - GAP_HI + 1)
    put("shvB", np.where(gap, 0.0, shiftval - BIGL))
    put("winidx", np.arange(R)[:, None] * PITCH
        + np.arange(CROP)[None, :])                               # [64, 256]
    return c


def _b3(ap, n, inner):
    """[p, inner] const AP -> [p, n, inner] broadcast over middle dim."""
    return ap.rearrange("p (a b) -> p a b", a=1).to_broadcast(
        [ap.shape[0], n, inner])


def _cmul_psum(nc, pool, eng, outr, outi, pr, pi, twr, twi, inner, n,
               part=128, tag="twtmp"):
    """(outr + i outi) = (pr + i pi) * (twr + i twi); p* in PSUM, tw const APs
    broadcast over n blocks of `inner`. outr/outi are SBUF APs [part, n*inner].
    eng: engine proxy with tensor_tensor (nc.vector or nc.gpsimd)."""
    tmp = pool.tile([part, n * inner], F32, tag=tag, name=tag)
    orv = outr.rearrange("p (a b) -> p a b", b=inner)
    oiv = outi.rearrange("p (a b) -> p a b", b=inner)
    prv = pr.rearrange("p (a b) -> p a b", b=inner)
    piv = pi.rearrange("p (a b) -> p a b", b=inner)
    tv = tmp[:].rearrange("p (a b) -> p a b", b=inner)
    eng.tensor_tensor(orv, prv, twr, op=AT.mult)
    eng.tensor_tensor(tv, piv, twi, op=AT.mult)
    eng.tensor_tensor(orv, orv, tv, op=AT.subtract)
    eng.tensor_tensor(oiv, prv, twi, op=AT.mult)
    eng.tensor_tensor(tv, piv, twr, op=AT.mult)
    eng.tensor_tensor(oiv, oiv, tv, op=AT.add)


def _spectral(nc, eng, sr, si, ar, ai, br, bi, tmp):
    """S = A * conj(B): sr = ar*br + ai*bi; si = ai*br - ar*bi."""
    eng.tensor_tensor(sr, ar, br, op=AT.mult)
    eng.tensor_tensor(tmp, ai, bi, op=AT.mult)
    eng.tensor_tensor(sr, sr, tmp, op=AT.add)
    eng.tensor_tensor(si, ai, br, op=AT.mult)
    eng.tensor_tensor(tmp, ar, bi, op=AT.mult)
    eng.tensor_tensor(si, si, tmp, op=AT.subtract)


def _mm_const_names():
    s = {"W1r", "W1i", "nW1i", "W1r65", "W1i65", "V2Ar", "V2Ai", "nV2Ai",
         "V1Ar", "nV1Ai", "V1Br", "nV1Bi"}
    s |= {f"W2Br{a}{b}" for a in range(2) for b in range(2)}
    s |= {f"W2Bi{a}{b}" for a in range(2) for b in range(2)}
    s |= {f"nW2Bi{a}{b}" for a in range(2) for b in range(2)}
    s |= {f"V2Br{a}" for a in range(2)} | {f"V2Bi{a}" for a in range(2)}
    s |= {f"nV2Bi{a}" for a in range(2)}
    return s


def build_nc(cdt=BF16, rows=R, rbb=8, rb2=4):
    nc = bacc.Bacc("TRN2", target_bir_lowering=False, debug=False,
                   num_devices=NCORES)
    consts = make_consts()

    pred = nc.dram_tensor("pred", [rows, L1], F32, kind="ExternalInput")
    true_ = nc.dram_tensor("true", [rows, L1], F32, kind="ExternalInput")
    egf = nc.dram_tensor("egf", [rows, L2], F32, kind="ExternalInput")
    target = nc.dram_tensor("target", [rows, CONV_LEN], F32,
                            kind="ExternalInput")
    out = nc.dram_tensor("out", [1, 2], F32, kind="ExternalOutput")
    scratch = nc.dram_tensor("scratch", [rows, PITCH], F32)

    MM_CONST = _mm_const_names()

    cdram = {}
    for name, arr in consts.items():
        cdt_n = cdt if name in MM_CONST else F32
        cdram[name] = nc.dram_tensor(name, list(arr.shape), cdt_n,
                                     kind="ExternalInput")

    nb1, nb2 = rows // rbb, rows // rb2

    with tile.TileContext(nc) as tc:
        with (
            tc.tile_pool(name="consts", bufs=1) as cpool,
            tc.tile_pool(name="keep", bufs=1) as kpool,
            tc.tile_pool(name="ps", bufs=2, space="PSUM") as pp,
        ):
            cs = {}
            for name, arr in consts.items():
                dt = cdt if name in MM_CONST else F32
                t = cpool.tile(list(arr.shape), dt, tag=f"c_{name}", name=f"c_{name}")
                nc.sync.dma_start(t[:], cdram[name][:])
                cs[name] = t

            allmax = kpool.tile([128, rows], F32, tag="allmax", name="allmax")
            allmin = kpool.tile([128, rows], F32, tag="allmin", name="allmin")
            ccm_all = kpool.tile([128, rows * 256], BF16, tag="ccm", name="ccm")
            astf_acc = kpool.tile([128, 8], F32, tag="astfacc", name="astfacc")
            shifts = kpool.tile([rows, 1], F32, tag="shifts", name="shifts")
            outt = kpool.tile([1, 2], F32, tag="outt", name="outt")

            # ---------------- A) astf ----------------
            predf = pred.ap().rearrange("r l -> (r l)").rearrange(
                "(p f) -> p f", p=128)
            truef = true_.ap().rearrange("r l -> (r l)").rearrange(
                "(p f) -> p f", p=128)
            fch = rows * L1 // 128 // 8
            with tc.tile_pool(name="astf", bufs=2) as apool:
                for i in range(8):
                    tp = apool.tile([128, fch], F32, tag="ap", name="ap")
                    tt = apool.tile([128, fch], F32, tag="at", name="at")
                    sl = bass.ts(i, fch)
                    nc.sync.dma_start(tp[:], predf[:, sl])
                    nc.sync.dma_start(tt[:], truef[:, sl])
                    nc.vector.tensor_tensor(tt[:], tp[:], tt[:], op=AT.subtract)
                    nc.vector.scalar_tensor_tensor(
                        tp[:], tt[:], 1.0, tt[:], op0=AT.bypass, op1=AT.mult,
                        accum_out=astf_acc[:, i:i + 1])

            # ---------------- B) 16K level ----------------
            with tc.tile_pool(name="p16", bufs=2) as dp:
                for b in range(nb1):
                    r0 = b * rbb
                    Dp = dp.tile([128, rbb * 128], cdt, tag="Dp", name="Dp")
                    De = dp.tile([16, rbb * 128], cdt, tag="De", name="De")
                    psrc = pred[r0:r0 + rbb, :].rearrange(
                        "q (a b) -> q a b", a=128).transpose([1, 0, 2])
                    esrc = egf[r0:r0 + rbb, :].rearrange(
                        "q (a b) -> q a b", a=16).transpose([1, 0, 2])
                    if cdt == F32:
                        nc.sync.dma_start(
                            Dp[:].rearrange("p (q b) -> p q b", b=128), psrc)
                        nc.sync.dma_start(
                            De[:].rearrange("p (q b) -> p q b", b=128), esrc)
                    else:
                        Dst = dp.tile([128, rbb * 128], F32, tag="Dst", name="Dst")
                        Est = dp.tile([16, rbb * 128], F32, tag="Est", name="Est")
                        nc.sync.dma_start(
                            Dst[:].rearrange("p (q b) -> p q b", b=128), psrc)
                        nc.sync.dma_start(
                            Est[:].rearrange("p (q b) -> p q b", b=128), esrc)
                        nc.scalar.copy(Dp[:], Dst[:])
                        nc.scalar.copy(De[:], Est[:])

                    Bs = {k: dp.tile([128, rbb * F1], cdt, tag=f"B{k}", name=f"B{k}")
                          for k in ("pr", "pi", "er", "ei")}
                    for g in range(rbb // 4):
                        gsl = bass.ts(g, 4 * F1)
                        for inp, D, kp in (("p", Dp, 128), ("e", De, 16)):
                            pa = pp.tile([128, 4 * F1], F32, tag="st1", name="st1")
                            pai = pp.tile([128, 4 * F1], F32, tag="st1", name="st1")
                            for q in range(4):
                                qq = g * 4 + q
                                sl, osl = bass.ts(qq, 128), bass.ts(q, F1)
                                nc.tensor.matmul(pa[:, osl], lhsT=D[:, sl],
                                                 rhs=cs["W1r65"][0:kp, :],
                                                 start=True, stop=True)
                                nc.tensor.matmul(pai[:, osl], lhsT=D[:, sl],
                                                 rhs=cs["W1i65"][0:kp, :],
                                                 start=True, stop=True)
                            _cmul_psum(nc, dp, nc.vector,
                                       Bs[inp + "r"][:, gsl], Bs[inp + "i"][:, gsl],
                                       pa[:], pai[:],
                                       _b3(cs["twAr"][:], 4, F1),
                                       _b3(cs["twAi"][:], 4, F1), F1, 4,
                                       tag="twtA")

                    Zs = {k: dp.tile([128, rbb * F1], cdt, tag=f"Z{k}", name=f"Z{k}")
                          for k in ("pr", "pi", "er", "ei")}
                    for g in range(rbb // 4):
                        gsl = bass.ts(g, 4 * F1)
                        for inp in ("p", "e"):
                            br, bi = Bs[inp + "r"], Bs[inp + "i"]
                            pzr = pp.tile([128, 4 * F1], F32, tag="st2", name="st2")
                            pzi = pp.tile([128, 4 * F1], F32, tag="st2", name="st2")
                            nc.tensor.matmul(pzr[:], lhsT=cs["W1r"][:],
                                             rhs=br[:, gsl], start=True, stop=False)
                            nc.tensor.matmul(pzr[:], lhsT=cs["nW1i"][:],
                                             rhs=bi[:, gsl], start=False, stop=True)
                            nc.tensor.matmul(pzi[:], lhsT=cs["W1i"][:],
                                             rhs=br[:, gsl], start=True, stop=False)
                            nc.tensor.matmul(pzi[:], lhsT=cs["W1r"][:],
                                             rhs=bi[:, gsl], start=False, stop=True)
                            nc.scalar.copy(Zs[inp + "r"][:, gsl], pzr[:])
                            nc.scalar.copy(Zs[inp + "i"][:, gsl], pzi[:])

                    Sr = dp.tile([128, rbb * F1], cdt, tag="Sr", name="Sr")
                    Si = dp.tile([128, rbb * F1], cdt, tag="Si", name="Si")
                    tmpb = dp.tile([128, rbb * F1], F32, tag="tmpbig", name="tmpbig")
                    _spectral(nc, nc.gpsimd, Sr[:], Si[:],
                              Zs["pr"][:], Zs["pi"][:], Zs["er"][:], Zs["ei"][:],
                              tmpb[:])

                    Hr = dp.tile([F1, rbb * 128], cdt, tag="Hr", name="Hr")
                    Hi = dp.tile([F1, rbb * 128], cdt, tag="Hi", name="Hi")
                    for g in range(rbb // 4):
                        gsl = bass.ts(g, 512)
                        pgr = pp.tile([F1, 512], F32, tag="inv", name="inv")
                        pgi = pp.tile([F1, 512], F32, tag="inv", name="inv")
                        for q in range(4):
                            qq = g * 4 + q
                            sl, osl = bass.ts(qq, F1), bass.ts(q, 128)
                            nc.tensor.matmul(pgr[:, osl], lhsT=Sr[:, sl],
                                             rhs=cs["V2Ar"][:], start=True, stop=False)
                            nc.tensor.matmul(pgr[:, osl], lhsT=Si[:, sl],
                                             rhs=cs["nV2Ai"][:], start=False, stop=True)
                            nc.tensor.matmul(pgi[:, osl], lhsT=Sr[:, sl],
                                             rhs=cs["V2Ai"][:], start=True, stop=False)
                            nc.tensor.matmul(pgi[:, osl], lhsT=Si[:, sl],
                                             rhs=cs["V2Ar"][:], start=False, stop=True)
                        Gr = dp.tile([F1, 512], cdt, tag="GrA", name="GrA")
                        Gi = dp.tile([F1, 512], cdt, tag="GiA", name="GiA")
                        nc.scalar.copy(Gr[:], pgr[:])
                        nc.scalar.copy(Gi[:], pgi[:])
                        _cmul_psum(nc, dp, nc.gpsimd, Hr[:, gsl], Hi[:, gsl],
                                   Gr[:], Gi[:],
                                   _b3(cs["itwAr"][:], 4, 128),
                                   _b3(cs["itwAi"][:], 4, 128), 128, 4,
                                   part=F1, tag="twtiA")

                    convSB = dp.tile([128, rbb * 128], F32, tag="convSB", name="convSB")
                    for g in range(rbb // 4):
                        gsl = bass.ts(g, 512)
                        pc = pp.tile([128, 512], F32, tag="cc", name="cc")
                        nc.tensor.matmul(pc[:], lhsT=cs["V1Ar"][:],
                                         rhs=Hr[:, gsl], start=True, stop=False)
                        nc.tensor.matmul(pc[:], lhsT=cs["nV1Ai"][:],
                                         rhs=Hi[:, gsl], start=False, stop=True)
                        nc.scalar.copy(convSB[:, gsl], pc[:])

                    nc.sync.dma_start(
                        scratch[r0:r0 + rbb, 0:14336].rearrange(
                            "q (a b) -> q a b", a=112).transpose([1, 0, 2]),
                        convSB[0:112, :].rearrange("p (q b) -> p q b", b=128))
                    nc.sync.dma_start(
                        scratch[r0:r0 + rbb, 14336:14337].rearrange("q x -> x q"),
                        convSB[112:113, 0:rbb * 128:128])
                    nc.sync.dma_start(
                        scratch[r0:r0 + rbb, 14337:14593].rearrange(
                            "q (a b) -> q a b", a=2).transpose([1, 0, 2]),
                        convSB[0:2, :].rearrange("p (q b) -> p q b", b=128))

            # ---------------- C) 32K level ----------------
            with tc.tile_pool(name="p32", bufs=2) as dp:
                for b in range(nb2):
                    r0 = b * rb2
                    D2c = dp.tile([57, rb2 * 256], cdt, tag="D2c", name="D2c")
                    D2t = dp.tile([57, rb2 * 256], cdt, tag="D2t", name="D2t")
                    if cdt == F32:
                        tgc, tgt_ = D2c, D2t
                    else:
                        tgc = dp.tile([57, rb2 * 256], F32, tag="D2cs", name="D2cs")
                        tgt_ = dp.tile([57, rb2 * 256], F32, tag="D2ts", name="D2ts")
                    nc.scalar.memzero(tgc[:])
                    nc.scalar.memzero(tgt_[:])
                    nc.sync.dma_start(
                        tgc[0:56, :].rearrange("p (q b) -> p q b", b=256),
                        scratch[r0:r0 + rb2, 0:14336].rearrange(
                            "q (a b) -> q a b", a=56).transpose([1, 0, 2]))
                    nc.sync.dma_start(
                        tgc[56:57, 0:rb2 * 256:256],
                        scratch[r0:r0 + rb2, 14336:14337].rearrange("q x -> x q"))
                    nc.sync.dma_start(
                        tgt_[0:56, :].rearrange("p (q b) -> p q b", b=256),
                        target[r0:r0 + rb2, 0:14336].rearrange(
                            "q (a b) -> q a b", a=56).transpose([1, 0, 2]))
                    nc.sync.dma_start(
                        tgt_[56:57, 0:rb2 * 256:256],
                        target[r0:r0 + rb2, 14336:14337].rearrange("q x -> x q"))
                    if cdt != F32:
                        nc.scalar.copy(D2c[:], tgc[:])
                        nc.scalar.copy(D2t[:], tgt_[:])

                    B2 = {}
                    for c in range(2):
                        for inp, D in (("c", D2c), ("t", D2t)):
                            br = dp.tile([128, rb2 * F1], cdt, tag=f"B2r{c}{inp}", name=f"B2r{c}{inp}")
                            bi = dp.tile([128, rb2 * F1], cdt, tag=f"B2i{c}{inp}", name=f"B2i{c}{inp}")
                            pa = pp.tile([128, rb2 * F1], F32, tag="st1", name="st1")
                            pai = pp.tile([128, rb2 * F1], F32, tag="st1", name="st1")
                            for q in range(rb2):
                                dsl = slice(q * 256 + c * 128,
                                            q * 256 + c * 128 + 128)
                                osl = bass.ts(q, F1)
                                nc.tensor.matmul(pa[:, osl], lhsT=D[:, dsl],
                                                 rhs=cs["W1r65"][0:57, :],
                                                 start=True, stop=True)
                                nc.tensor.matmul(pai[:, osl], lhsT=D[:, dsl],
                                                 rhs=cs["W1i65"][0:57, :],
                                                 start=True, stop=True)
                            _cmul_psum(nc, dp, nc.vector, br[:], bi[:],
                                       pa[:], pai[:],
                                       _b3(cs[f"twBr{c}"][:], rb2, F1),
                                       _b3(cs[f"twBi{c}"][:], rb2, F1), F1, rb2,
                                       tag="twtB")
                            B2[(c, inp)] = (br, bi)

                    Z2 = {}
                    for inp in ("c", "t"):
                        for f2c in range(2):
                            zr = dp.tile([128, rb2 * F1], cdt, tag=f"Z2r{inp}{f2c}", name=f"Z2r{inp}{f2c}")
                            zi = dp.tile([128, rb2 * F1], cdt, tag=f"Z2i{inp}{f2c}", name=f"Z2i{inp}{f2c}")
                            pzr = pp.tile([128, rb2 * F1], F32, tag="st2", name="st2")
                            pzi = pp.tile([128, rb2 * F1], F32, tag="st2", name="st2")
                            for t2c in range(2):
                                br, bi = B2[(t2c, inp)]
                                nc.tensor.matmul(pzr[:], lhsT=cs[f"W2Br{t2c}{f2c}"][:],
                                                 rhs=br[:], start=(t2c == 0), stop=False)
                                nc.tensor.matmul(pzr[:], lhsT=cs[f"nW2Bi{t2c}{f2c}"][:],
                                                 rhs=bi[:], start=False, stop=(t2c == 1))
                                nc.tensor.matmul(pzi[:], lhsT=cs[f"W2Bi{t2c}{f2c}"][:],
                                                 rhs=br[:], start=(t2c == 0), stop=False)
                                nc.tensor.matmul(pzi[:], lhsT=cs[f"W2Br{t2c}{f2c}"][:],
                                                 rhs=bi[:], start=False, stop=(t2c == 1))
                            nc.scalar.copy(zr[:], pzr[:])
                            nc.scalar.copy(zi[:], pzi[:])
                            Z2[(inp, f2c)] = (zr, zi)

                    S2 = {}
                    tmpc = dp.tile([128, rb2 * F1], F32, tag="tmpc", name="tmpc")
                    for f2c in range(2):
                        zcr, zci = Z2[("c", f2c)]
                        ztr, zti = Z2[("t", f2c)]
                        sr = dp.tile([128, rb2 * F1], cdt, tag=f"S2r{f2c}", name=f"S2r{f2c}")
                        si = dp.tile([128, rb2 * F1], cdt, tag=f"S2i{f2c}", name=f"S2i{f2c}")
                        _spectral(nc, nc.gpsimd, sr[:], si[:], zcr[:], zci[:],
                                  ztr[:], zti[:], tmpc[:])
                        S2[f2c] = (sr, si)

                    H2r = dp.tile([F1, rb2 * 256], cdt, tag="H2r", name="H2r")
                    H2i = dp.tile([F1, rb2 * 256], cdt, tag="H2i", name="H2i")
                    for g in range(rb2 // 2):
                        pgr = pp.tile([F1, 512], F32, tag="inv", name="inv")
                        pgi = pp.tile([F1, 512], F32, tag="inv", name="inv")
                        for q in range(2):
                            qq = g * 2 + q
                            sl, osl = bass.ts(qq, F1), bass.ts(q, 256)
                            for f2c in range(2):
                                sr, si = S2[f2c]
                                nc.tensor.matmul(pgr[:, osl], lhsT=sr[:, sl],
                                                 rhs=cs[f"V2Br{f2c}"][:],
                                                 start=(f2c == 0), stop=False)
                                nc.tensor.matmul(pgr[:, osl], lhsT=si[:, sl],
                                                 rhs=cs[f"nV2Bi{f2c}"][:],
                                                 start=False, stop=(f2c == 1))
                                nc.tensor.matmul(pgi[:, osl], lhsT=sr[:, sl],
                                                 rhs=cs[f"V2Bi{f2c}"][:],
                                                 start=(f2c == 0), stop=False)
                                nc.tensor.matmul(pgi[:, osl], lhsT=si[:, sl],
                                                 rhs=cs[f"V2Br{f2c}"][:],
                                                 start=False, stop=(f2c == 1))
                        gsl = bass.ts(g, 512)
                        G2r = dp.tile([F1, 512], cdt, tag="G2r", name="G2r")
                        G2i = dp.tile([F1, 512], cdt, tag="G2i", name="G2i")
                        nc.scalar.copy(G2r[:], pgr[:])
                        nc.scalar.copy(G2i[:], pgi[:])
                        _cmul_psum(nc, dp, nc.gpsimd, H2r[:, gsl], H2i[:, gsl],
                                   G2r[:], G2i[:],
                                   _b3(cs["itwBr"][:], 2, 256),
                                   _b3(cs["itwBi"][:], 2, 256), 256, 2,
                                   part=F1, tag="twtiB")

                    for g in range(rb2 // 2):
                        gsl = bass.ts(g, 512)
                        pcc = pp.tile([128, 512], F32, tag="cc", name="cc")
                        nc.tensor.matmul(pcc[:], lhsT=cs["V1Br"][:],
                                         rhs=H2r[:, gsl], start=True, stop=False)
                        nc.tensor.matmul(pcc[:], lhsT=cs["nV1Bi"][:],
                                         rhs=H2i[:, gsl], start=False, stop=True)
                        csl = slice((r0 + g * 2) * 256, (r0 + g * 2 + 2) * 256)
                        ccv = ccm_all[:, csl].rearrange("p (a b) -> p a b", b=256)
                        nc.vector.scalar_tensor_tensor(
                            ccv, pcc[:].rearrange("p (a b) -> p a b", b=256),
                            1.0, _b3(cs["maskB"][:], 2, 256),
                            op0=AT.bypass, op1=AT.add)
                        nc.vector.tensor_reduce(
                            allmax[:, r0 + g * 2:r0 + g * 2 + 2], ccv,
                            axis=AX.X, op=AT.max)

            # ---------------- D) argmax -> shifts ----------------
            with tc.tile_pool(name="amax", bufs=1) as dp:
                pt = pp.tile([rows, 128], F32, tag="st1", name="st1")
                nc.tensor.transpose(pt[:], allmax[:, 0:rows], cs["ident"][:])
                tmax = dp.tile([rows, 128], F32, tag="tmax", name="tmax")
                nc.scalar.copy(tmax[:], pt[:])
                rowmax = dp.tile([rows, 1], F32, tag="rowmax", name="rowmax")
                nc.vector.tensor_reduce(rowmax[:], tmax[:], axis=AX.X, op=AT.max)
                prm = pp.tile([1, rows], F32, tag="st2", name="st2")
                nc.tensor.transpose(prm[:], rowmax[:], cs["ident"][0:rows, 0:rows])
                rmT = dp.tile([1, rows], F32, tag="rmT", name="rmT")
                nc.scalar.copy(rmT[:], prm[:])
                pmb = pp.tile([128, rows], F32, tag="inv", name="inv")
                nc.tensor.matmul(pmb[:], lhsT=cs["ones1x128"][:], rhs=rmT[:],
                                 start=True, stop=True)
                Mb = dp.tile([128, rows], F32, tag="Mb", name="Mb")
                nc.scalar.copy(Mb[:], pmb[:])

                eqm = dp.tile([128, min(rows, 8) * 256], BF16, tag="eqm", name="eqm")
                selm = dp.tile([128, min(rows, 8) * 256], F32, tag="selm", name="selm")
                for bb in range(max(1, rows // 8)):
                    csl = bass.ts(bb, min(rows, 8) * 256)
                    nr8 = min(rows, 8)
                    mbb = Mb[:, bb * nr8:(bb + 1) * nr8]\
                        .rearrange("p (a b) -> p a b", b=1)\
                        .to_broadcast([128, nr8, 256])
                    ccv = ccm_all[:, csl].rearrange("p (a b) -> p a b", b=256)
                    nc.vector.tensor_tensor(
                        eqm[:].rearrange("p (a b) -> p a b", b=256),
                        ccv, mbb, op=AT.is_equal)
                    nc.vector.tensor_tensor(
                        selm[:].rearrange("p (a b) -> p a b", b=256),
                        eqm[:].rearrange("p (a b) -> p a b", b=256),
                        _b3(cs["shvB"][:], nr8, 256), op=AT.mult)
                    nc.vector.tensor_reduce(
                        allmin[:, bb * nr8:(bb + 1) * nr8],
                        selm[:].rearrange("p (a b) -> p a b", b=256),
                        axis=AX.X, op=AT.min)
                pt2 = pp.tile([rows, 128], F32, tag="cc", name="cc")
                nc.tensor.transpose(pt2[:], allmin[:, 0:rows], cs["ident"][:])
                tmin = dp.tile([rows, 128], F32, tag="tmin", name="tmin")
                nc.scalar.copy(tmin[:], pt2[:])
                nc.vector.tensor_reduce(shifts[:], tmin[:], axis=AX.X, op=AT.min)
                nc.vector.tensor_scalar_add(shifts[:], shifts[:], BIGL + float(START0))

                # start = (7040 + shift) mod 14337
                m1 = dp.tile([rows, 1], F32, tag="m1", name="m1")
                nc.vector.tensor_scalar(out=m1[:], in0=shifts[:], scalar1=0.0,
                                        scalar2=None, op0=AT.is_lt)
                nc.vector.scalar_tensor_tensor(
                    shifts[:], m1[:], float(CONV_LEN), shifts[:],
                    op0=AT.mult, op1=AT.add)
                nc.vector.tensor_scalar(out=m1[:], in0=shifts[:],
                                        scalar1=float(CONV_LEN), scalar2=None,
                                        op0=AT.is_ge)
                nc.vector.scalar_tensor_tensor(
                    shifts[:], m1[:], float(-CONV_LEN), shifts[:],
                    op0=AT.mult, op1=AT.add)

                idxf = dp.tile([rows, CROP], F32, tag="idxf", name="idxf")
                nc.vector.tensor_tensor(idxf[:], cs["winidx"][0:rows, :],
                                        shifts[:].to_broadcast([rows, CROP]),
                                        op=AT.add)
                idxi = dp.tile([rows, CROP], I32, tag="idxi", name="idxi")
                nc.vector.tensor_copy(idxi[:], idxf[:])
                w = dp.tile([rows, CROP], F32, tag="wg", name="wg")
                nc.gpsimd.indirect_dma_start(
                    out=w[:], out_offset=None,
                    in_=scratch.ap().rearrange("r p -> (r p)").rearrange(
                        "(a b) -> a b", b=1),
                    in_offset=bass.IndirectOffsetOnAxis(ap=idxi[:], axis=0),
                )
                tw_ = dp.tile([rows, CROP], F32, tag="twin", name="twin")
                nc.sync.dma_start(tw_[:], target[:, START0:START0 + CROP])
                nc.vector.tensor_tensor(w[:], w[:], tw_[:], op=AT.subtract)
                convacc = dp.tile([rows, 1], F32, tag="convacc", name="convacc")
                nc.vector.scalar_tensor_tensor(
                    tw_[:], w[:], 1.0, w[:], op0=AT.bypass, op1=AT.mult,
                    accum_out=convacc[:])

                a0 = dp.tile([128, 1], F32, tag="a0", name="a0")
                nc.vector.tensor_reduce(a0[:], astf_acc[:], axis=AX.X, op=AT.add)
                psa = pp.tile([1, 1], F32, tag="st1", name="st1")
                nc.tensor.matmul(psa[:], lhsT=a0[:], rhs=cs["ones128"][:],
                                 start=True, stop=True)
                psc = pp.tile([1, 1], F32, tag="st2", name="st2")
                nc.tensor.matmul(psc[:], lhsT=convacc[:], rhs=cs["ones64"][0:rows, :],
                                 start=True, stop=True)
                nc.scalar.copy(outt[:, 0:1], psa[:])
                nc.scalar.copy(outt[:, 1:2], psc[:])
                nc.sync.dma_start(out[:], outt[:])

    nc.finalize()
    return nc, consts


_CACHE = {}


def get_built(cdt=BF16):
    key = str(cdt)
    if key not in _CACHE:
        _CACHE[key] = build_nc(cdt=cdt)
    return _CACHE[key]


LAST_RESULT = {}


def kernel(pred_astf, true_astf, egf, target_waveform):
    import os
    from concourse.bass_utils import run_bass_kernel_spmd
    cdt = F32 if os.environ.get("CONVALIGN_F32") == "1" else BF16
    nc, consts = get_built(cdt)
    if cdt != F32:
        import ml_dtypes
        mmnames = _mm_const_names()
        consts = {k: (v.astype(ml_dtypes.bfloat16) if k in mmnames else v)
                  for k, v in consts.items()}
    pred_astf = np.ascontiguousarray(np.asarray(pred_astf, np.float32))
    true_astf = np.ascontiguousarray(np.asarray(true_astf, np.float32))
    egf = np.ascontiguousarray(np.asarray(egf, np.float32))
    target_waveform = np.ascontiguousarray(
        np.asarray(target_waveform, np.float32))
    B = pred_astf.shape[0]
    per = B // NCORES
    in_maps = []
    for i in range(NCORES):
        sl = slice(i * per, (i + 1) * per)
        m = {"pred": pred_astf[sl], "true": true_astf[sl],
             "egf": egf[sl], "target": target_waveform[sl]}
        m.update(consts)
        in_maps.append(m)
    trace = os.environ.get("CONVALIGN_TRACE") == "1"
    res = run_bass_kernel_spmd(nc, in_maps, core_ids=list(range(NCORES)),
                               trace=trace)
    LAST_RESULT["res"] = res
    sums = np.stack([res.results[i]["out"][0] for i in range(NCORES)])
    loss_astf = np.float32(sums[:, 0].sum() / (B * L1))
    loss_conv = np.float32(sums[:, 1].sum() / (B * CROP))
    total = np.float32(loss_astf + loss_conv)
    return total, loss_astf, loss_conv
